# revision 5
# baseline (speedup 1.0000x reference)
"""Multi-head causal attention (B=4, T=2048, E=1024, H=16, D=64) on 8 trn2
NeuronCores via Bass/Tile.

Sharding: core c handles batch b = c//2 and heads [half*8, half*8+8), half =
c%2. Each core computes its 8 heads' attention and a partial output
projection; the host sums the two half partials per batch, transposes, and
adds the bias.

On-device layout is "transposed": activations are [feature, token] so every
matmul contracts over the partition dim. Softmax denominators come from a
ones-column appended to the stationary V operand (M=65 matmuls); masking is
applied block-wise (128x128) with patterns derived from the actual mask input
at build time. No max-subtraction is needed: scores are ~N(0, 0.083^2).

This version software-pipelines the whole kernel: the attention i-loop is
ACT(exp)-paced, so projection matmuls for the next t-tile, output-projection
matmuls for the previous t-tile, and softmax-tail work are injected as
"filler" closures between attention steps to keep the PE busy. Inputs are
pre-tiled host-side so every DMA moves one contiguous 128KB block.
"""
import numpy as np
import ml_dtypes
from collections import deque
from contextlib import ExitStack

import concourse.bass as bass
import concourse.mybir as mybir
import concourse.tile as tile
from concourse.bass_utils import run_bass_kernel_spmd
from concourse.vector_clock import ScopedClock

BF16 = mybir.dt.bfloat16
F32 = mybir.dt.float32
NPBF16 = ml_dtypes.bfloat16

B, T, E, H, D = 4, 2048, 1024, 16, 64
HPC = 8            # heads per core
DC = HPC * D       # 512: stacked head dim per core
TJ = 512           # t tile (matmul free dim)
NJ = T // TJ       # 4
SI = 128           # s tile (psum partition dim)
NSI = T // SI      # 16
EC = E // 128      # 8 e-chunks
NP = HPC // 2      # 4 head pairs

# ---------------------------------------------------------------------------
# Workarounds for this walrus build: at most ONE sync wait per instruction.
# ---------------------------------------------------------------------------
_PATCHED = False


def _patched_drain_and_barrier(self, tick_clock, wait_clock):
    drain_inst = self.nc.sync.drain(fusable=False)
    wait_clock.add_sem_waits(
        drain_inst.ins, ScopedClock({None: tick_clock.global_clock})
    )
    si = drain_inst.ins.sync_info
    if si is not None and len(si.on_wait) > 1:
        waits = list(si.on_wait)
        drain_inst.ins.sync_info = mybir.SyncInfo(
            on_wait=waits[:1], on_update=list(si.on_update)
        )
        for ofs in range(1, len(waits)):
            extra = self.nc.sync.drain(fusable=False)
            extra.ins.sync_info = mybir.SyncInfo(
                on_wait=waits[ofs : ofs + 1], on_update=[]
            )
    self.nc.all_engine_barrier()
    assert self.sems is not None
    popped = self.nc._tile_sem_poison_stack.pop()
    assert popped is self._sem_poison
    self.nc.clear_and_free_semaphores(list(self.sems.allocated().values()))
    self.nc.all_engine_barrier()


def _install_patches():
    global _PATCHED
    if _PATCHED:
        return
    tile.TileContext._drain_and_barrier = _patched_drain_and_barrier
    _PATCHED = True


def _make_carrier(nc, engine, wait):
    """Wait-only EventSemaphore on `engine` (cheap: ~70ns, no pipe flush)."""
    ev = mybir.InstEventSemaphore(name=f"W-{nc.next_id()}", ins=[], outs=[])
    ev.engine = engine
    ev.sync_info = mybir.SyncInfo(on_wait=[wait], on_update=[])
    return ev


_ENGINE_SEM = {
    "EngineType.PE": "PE",
    "EngineType.DVE": "DVE",
    "EngineType.Activation": "Activation",
    "EngineType.SP": "SP",
    "EngineType.Pool": "Pool",
}
# engines with in-order issue AND in-order completion for these inst types:
# a wait on the engine's own completion sem is redundant. Ldweights excluded
# (the PE reorder window pulls it ahead of in-flight matmuls).
_DROPPABLE = (
    "InstMatmult", "InstActivation", "InstTensorTensor", "InstTensorCopy",
    "InstTensorReduce", "InstMemset", "InstReciprocal", "InstDMACopy",
    "InstCopyPredicated", "InstTensorScalarPtr", "InstTensorScalar",
    "InstCast", "InstDveOp", "InstCustomDve",
)


def _split_multi_waits(nc):
    for bbw in list(nc.bb_map.values()):
        bb = bbw.bb
        insts = bb.instructions
        if not any(
            getattr(i, "sync_info", None) is not None and len(i.sync_info.on_wait) > 1
            for i in insts
        ):
            continue
        out = []
        for inst in insts:
            si = getattr(inst, "sync_info", None)
            waits = list(si.on_wait) if si is not None else []
            if len(waits) > 1:
                own = _ENGINE_SEM.get(str(inst.engine))
                tn = type(inst).__name__
                if own is not None and tn.startswith(_DROPPABLE):
                    waits = [
                        w for w in waits
                        if w.ant_name.rsplit("_", 1)[0] != own
                    ] or waits[-1:]
            if len(waits) > 1:
                for w in waits[:-1]:
                    out.append(_make_carrier(nc, inst.engine, w))
                waits = waits[-1:]
            if si is not None and list(si.on_wait) != waits:
                inst.sync_info = mybir.SyncInfo(
                    on_wait=waits, on_update=list(si.on_update)
                )
            out.append(inst)
        insts[:] = out


# ---------------------------------------------------------------------------
# Mask analysis (host side, 128x128 blocks).
# ---------------------------------------------------------------------------
def _classify_mask(mask):
    """mask: [T, T] bool, mask[t, s]=True means masked (score -> -inf).

    Returns (btab, patterns): btab[i][jj] in {'skip', 'dense', int u};
    patterns[u] is a [128,128] bf16 multiplier in [s, t] orientation."""
    nb = T // 128
    m = np.asarray(mask, dtype=bool)
    patterns = []
    index = {}
    btab = [[None] * nb for _ in range(nb)]
    for i in range(nb):          # s block
        for jj in range(nb):     # t block
            sub = m[jj * 128 : (jj + 1) * 128, i * 128 : (i + 1) * 128]  # [t, s]
            if sub.all():
                btab[i][jj] = "skip"
            elif not sub.any():
                btab[i][jj] = "dense"
            else:
                pat = (~sub).T.astype(NPBF16)  # [s, t] multiplier
                key = pat.tobytes()
                if key not in index:
                    index[key] = len(patterns)
                    patterns.append(pat)
                btab[i][jj] = index[key]
    if not patterns:
        patterns.append(np.ones((128, 128), NPBF16))
    return btab, np.stack(patterns)


# ---------------------------------------------------------------------------
# Kernel builder (SPMD program, identical on all 8 cores).
# ---------------------------------------------------------------------------
def _build(btab, n_pat):
    nc = bass.Bass()
    # pre-tiled streams: tile (j, e) is rows (j*EC+e)*128 .. +128, contiguous
    qTt = nc.declare_dram_parameter("qTt", [NJ * EC * 128, TJ], BF16, isOutput=False)
    kTt = nc.declare_dram_parameter("kTt", [NJ * EC * 128, TJ], BF16, isOutput=False)
    vTt = nc.declare_dram_parameter("vTt", [NJ * EC * 128, TJ], BF16, isOutput=False)
    wq = nc.declare_dram_parameter("wq", [E, DC], BF16, isOutput=False)
    wk = nc.declare_dram_parameter("wk", [E, DC], BF16, isOutput=False)
    wv = nc.declare_dram_parameter("wv", [E, DC], BF16, isOutput=False)
    wpT = nc.declare_dram_parameter("wpT", [DC, E], BF16, isOutput=False)
    pat = nc.declare_dram_parameter("pat", [n_pat * 128, 128], BF16, isOutput=False)
    selbc = nc.declare_dram_parameter("selbc", [HPC, DC], BF16, isOutput=False)
    # pre-tiled output: tile (j, m) is rows (j*EC+m)*128 .. +128
    yTt = nc.declare_dram_parameter("yTt", [NJ * EC * 128, TJ], F32, isOutput=True)

    EXP = mybir.ActivationFunctionType.Exp

    with ExitStack() as ctx:
        tc = ctx.enter_context(tile.TileContext(nc))
        # SBUF pools
        consts = ctx.enter_context(tc.tile_pool(name="consts", bufs=1))
        streams = ctx.enter_context(tc.tile_pool(name="streams", bufs=1))
        acts = ctx.enter_context(tc.tile_pool(name="acts", bufs=1))
        work = ctx.enter_context(tc.tile_pool(name="work", bufs=1))
        # PSUM pools
        psA = ctx.enter_context(tc.tile_pool(name="psA", bufs=1, space="PSUM"))
        psB = ctx.enter_context(tc.tile_pool(name="psB", bufs=1, space="PSUM"))

        # ---- constants ----
        wq_sb = [consts.tile([128, DC], BF16, tag=f"wq{e}", name=f"wq{e}", bufs=1) for e in range(EC)]
        wk_sb = [consts.tile([128, DC], BF16, tag=f"wk{e}", name=f"wk{e}", bufs=1) for e in range(EC)]
        wv_sb = [consts.tile([128, DC], BF16, tag=f"wv{e}", name=f"wv{e}", bufs=1) for e in range(EC)]
        wp_sb = [consts.tile([128, E], BF16, tag=f"wp{p}", name=f"wp{p}", bufs=1) for p in range(NP)]
        pat_sb = [consts.tile([128, 128], BF16, tag=f"pat{u}", name=f"pat{u}", bufs=1) for u in range(n_pat)]
        selbc_sb = consts.tile([HPC, DC], BF16, tag="selbc", name="selbc", bufs=1)
        dummy_sb = consts.tile([1, 2], BF16, tag="dummy", name="dummy", bufs=1)

        # ---- persistent activations (per (p, j) tiles so pipelined writes
        # to tile j+1 never alias reads of tile j) ----
        xq_t = [[acts.tile([128, TJ], BF16, tag=f"xq{p}_{j}", name=f"xq{p}_{j}", bufs=1)
                 for j in range(NJ)] for p in range(NP)]
        xk_t = [[acts.tile([128, TJ], BF16, tag=f"xk{p}_{j}", name=f"xk{p}_{j}", bufs=1)
                 for j in range(NJ)] for p in range(NP)]
        # xv tiles: per s-tile, heads laid out as 8 x (64 cols xv | 1 col ones)
        xv_sb = [acts.tile([128, HPC * 65], BF16, tag=f"xv{i}", name=f"xv{i}", bufs=1) for i in range(NSI)]
        osc_sb_all = [
            [acts.tile([128, TJ], BF16, tag=f"osc{p}_{jj}", name=f"osc{p}_{jj}", bufs=1)
             for p in range(NP)]
            for jj in range(2)
        ]

        # ---------------------------------------------------------------
        # Filler queue: closures emitting ~1-2 PE matmuls (+ their DVE/DMA
        # tails). `require(key)` force-drains through a named closure.
        # ---------------------------------------------------------------
        fillers = deque()        # entries: (cost, fn)
        ready_idx = {}           # key -> push counter of last closure for key
        drained = [0]            # count of executed closures
        pushed = [0]
        debt = [0.0]

        def push(cost, fn, key=None):
            fillers.append((cost, fn))
            pushed[0] += 1
            if key is not None:
                ready_idx[key] = pushed[0]

        def _run_one():
            cost, fn = fillers.popleft()
            fn()
            drained[0] += 1
            return cost

        def pump(units):
            debt[0] += units
            while fillers and debt[0] >= fillers[0][0]:
                debt[0] -= _run_one()

        def require(key):
            idx = ready_idx.get(key, 0)
            while drained[0] < idx:
                _run_one()

        # ---------------------------------------------------------------
        # DMA emission
        # ---------------------------------------------------------------
        def dma_stream(dst_pool_tiles, src, j, e):
            r0 = (j * EC + e) * 128
            nc.sync.dma_start(out=dst_pool_tiles[e][:], in_=src[r0 : r0 + 128, :])

        def emit_stream_dmas(j):
            for e in range(EC):
                dma_stream(qs_j[j], qTt, j, e)
                dma_stream(ks_j[j], kTt, j, e)
                dma_stream(vs_j[j], vTt, j, e)

        # explicit per-j stream tile handles (2 buffer sets, alternating)
        qs_j, ks_j, vs_j = {}, {}, {}

        def alloc_stream_tiles(j):
            qs_j[j] = [streams.tile([128, TJ], BF16, tag=f"qs{e}", name=f"qs{e}_{j}", bufs=2) for e in range(EC)]
            ks_j[j] = [streams.tile([128, TJ], BF16, tag=f"ks{e}", name=f"ks{e}_{j}", bufs=2) for e in range(EC)]
            vs_j[j] = [streams.tile([128, TJ], BF16, tag=f"vs{e}", name=f"vs{e}_{j}", bufs=2) for e in range(EC)]

        # ---------------------------------------------------------------
        # Projection chain closures
        # ---------------------------------------------------------------
        def push_qk_chain(p, j):
            """xq and xk chains for (p, j): 4 closures of 4 matmuls."""
            pc = slice(p * 128, (p + 1) * 128)
            st_q = {}

            def q1():
                ps = psA.tile([128, TJ], F32, tag="mm512", bufs=2, name=f"xqp{p}_{j}")
                st_q['ps'] = ps
                for e in range(4):
                    nc.tensor.matmul(ps[:], wq_sb[e][:, pc], qs_j[j][e][:],
                                     start=(e == 0), stop=False)

            def q2():
                ps = st_q['ps']
                for e in range(4, EC):
                    nc.tensor.matmul(ps[:], wq_sb[e][:, pc], qs_j[j][e][:],
                                     start=False, stop=(e == EC - 1))
                nc.vector.tensor_copy(xq_t[p][j][:], ps[:])

            st_k = {}

            def k1():
                ps = psA.tile([128, TJ], F32, tag="mm512", bufs=2, name=f"xkp{p}_{j}")
                st_k['ps'] = ps
                for e in range(4):
                    nc.tensor.matmul(ps[:], wk_sb[e][:, pc], ks_j[j][e][:],
                                     start=(e == 0), stop=False)

            def k2():
                ps = st_k['ps']
                for e in range(4, EC):
                    nc.tensor.matmul(ps[:], wk_sb[e][:, pc], ks_j[j][e][:],
                                     start=False, stop=(e == EC - 1))
                nc.vector.tensor_copy(xk_t[p][j][:], ps[:])

            push(2, q1)
            push(2, q2, key=("xq", p, j))
            push(2, k1)
            push(2, k2, key=("xk", p, j))

        def push_xv_chain(loc, j):
            si = 4 * j + loc
            st = {}

            def v1():
                ps = psA.tile([128, DC], F32, tag="mm512", bufs=2, name=f"xvp{si}")
                st['ps'] = ps
                for e in range(4):
                    nc.tensor.matmul(ps[:], vs_j[j][e][:, loc * 128 : (loc + 1) * 128],
                                     wv_sb[e][:], start=(e == 0), stop=False)

            def v2():
                ps = st['ps']
                for e in range(4, EC):
                    nc.tensor.matmul(ps[:], vs_j[j][e][:, loc * 128 : (loc + 1) * 128],
                                     wv_sb[e][:], start=False, stop=(e == EC - 1))
                nc.vector.tensor_copy(
                    xv_sb[si][:].rearrange("p (h x) -> p h x", x=65)[:, :, 0:64],
                    ps[:].rearrange("p (h d) -> p h d", h=HPC),
                )

            push(2, v1)
            push(2, v2, key=("xv", si))

        def push_proj(j, first_p_inline=False):
            """All projection chains for t-tile j, p0's q/k first."""
            order = []
            if not first_p_inline:
                order.append(("qk", 0))
            order += [("xv", loc) for loc in range(4)]
            order += [("qk", p) for p in range(1, NP)]
            for kind, a in order:
                if kind == "qk":
                    push_qk_chain(a, j)
                else:
                    push_xv_chain(a, j)

        # ---------------------------------------------------------------
        # Softmax tail + output projection closures
        # ---------------------------------------------------------------
        def push_tail(j, rcat_sb, osb_sb, osc_sb):
            def recip():
                rr32 = work.tile([HPC, TJ], F32, tag="rrcat32", bufs=2, name=f"rrc32_{j}")
                nc.vector.reciprocal(rr32[:], rcat_sb[:])
                rr = work.tile([HPC, TJ], BF16, tag="rrcat", bufs=2, name=f"rrc_{j}")
                nc.vector.tensor_copy(rr[:], rr32[:])
                tail_state[j] = rr

            push(1, recip)

            for p in range(NP):
                def mk(p):
                    def rbosc():
                        rr = tail_state[j]
                        rb_ps = psA.tile([128, TJ], F32, tag="mm512", bufs=2,
                                         name=f"rb_{p}_{j}")
                        nc.tensor.matmul(
                            rb_ps[:], selbc_sb[:, p * 128 : (p + 1) * 128], rr[:],
                            start=True, stop=True,
                        )
                        nc.vector.tensor_mul(osc_sb[p][:], osb_sb[p][:], rb_ps[:])
                    return rbosc
                push(1, mk(p), key=("osc", p, j))

        def push_y(j, osc_sb):
            for m in range(EC):
                def mk(m):
                    def ychunk():
                        y_ps = psA.tile([128, TJ], F32, tag="mm512", bufs=2,
                                        name=f"y_{m}_{j}")
                        for p in range(NP):
                            nc.tensor.matmul(
                                y_ps[:], wp_sb[p][:, m * 128 : (m + 1) * 128],
                                osc_sb[p][:],
                                start=(p == 0), stop=(p == NP - 1),
                            )
                        y_sb = work.tile([128, TJ], F32, tag="y", bufs=2,
                                         name=f"ysb_{m}_{j}")
                        nc.vector.tensor_copy(y_sb[:], y_ps[:])
                        r0 = (j * EC + m) * 128
                        nc.sync.dma_start(out=yTt[r0 : r0 + 128, :], in_=y_sb[:])
                    return ychunk
                push(2, mk(m))

        tail_state = {}

        # ---------------------------------------------------------------
        # Attention cell for (p, j): ACT-paced i-loop with filler pumping
        # ---------------------------------------------------------------
        def emit_attention(p, j):
            jt = slice(j * TJ, (j + 1) * TJ)
            ivals = []
            for i in range(NSI):
                types = [btab[i][4 * j + bl] for bl in range(4)]
                if all(t == "skip" for t in types):
                    continue
                ivals.append((i, types))
            n_i = len(ivals)

            require(("xq", p, j))

            o_ps = [
                psB.tile([65, TJ], F32, tag=f"ops{hh}", name=f"ops{hh}_{p}_{j}", bufs=1)
                for hh in range(2)
            ]
            touched = [[False] * 4, [False] * 4]
            sts = [None] * n_i     # (st_tile, u_tile, c0)

            def emit_pair(k):
                i, types = ivals[k]
                c0 = next(bl for bl in range(4) if types[bl] != "skip")
                require(("xk", p, i // 4))
                st = psA.tile([128, 2 * TJ], F32, tag="st", bufs=2)
                for hh in range(2):
                    hr = slice(hh * 64, (hh + 1) * 64)
                    nc.tensor.matmul(
                        st[:, hh * TJ + c0 * 128 : (hh + 1) * TJ],
                        xk_t[p][i // 4][hr, (i % 4) * 128 : (i % 4 + 1) * 128],
                        xq_t[p][j][hr, c0 * 128 : TJ],
                        start=True, stop=True,
                    )
                sts[k] = (st, None, c0)

            def emit_exp(k):
                st, _, c0 = sts[k]
                u = work.tile([128, 2 * TJ], BF16, tag="u", bufs=4)
                nc.scalar.activation(
                    u[:].rearrange("p (g c) -> p g c", g=2)[:, :, c0 * 128 : TJ],
                    st[:].rearrange("p (g c) -> p g c", g=2)[:, :, c0 * 128 : TJ],
                    EXP, scale=1.0 / 32.0,
                )
                sts[k] = (st, u, c0)

            def emit_av(k):
                i, types = ivals[k]
                _, u, c0 = sts[k]
                require(("xv", i))
                for hh in range(2):
                    h = 2 * p + hh
                    uo = hh * TJ
                    runs = []  # (bl0, bl1, src_ap)
                    bl = c0
                    while bl < 4:
                        if types[bl] == "dense":
                            b2 = bl
                            while b2 + 1 < 4 and types[b2 + 1] == "dense":
                                b2 += 1
                            runs.append((bl, b2 + 1,
                                         u[:, uo + bl * 128 : uo + (b2 + 1) * 128]))
                            bl = b2 + 1
                        elif types[bl] == "skip":
                            bl += 1
                        else:
                            mt = work.tile([128, 128], BF16, tag="mfix", bufs=4)
                            nc.vector.tensor_mul(
                                mt[:], u[:, uo + bl * 128 : uo + (bl + 1) * 128],
                                pat_sb[types[bl]][:],
                            )
                            runs.append((bl, bl + 1, mt[:]))
                            bl += 1
                    lhs_v = xv_sb[i][:, h * 65 : h * 65 + 65]
                    for ri, (b0, b1, src) in enumerate(runs):
                        first = all(not touched[hh][b] for b in range(b0, b1))
                        assert first == any(
                            not touched[hh][b] for b in range(b0, b1)
                        ), "mask blocks: mixed touch state inside a run"
                        last = (k == n_i - 1) and (ri == len(runs) - 1)
                        nc.tensor.matmul(
                            o_ps[hh][:, b0 * 128 : b1 * 128],
                            lhs_v, src,
                            start=first, stop=last,
                            skip_group_check=True,
                        )
                        for b in range(b0, b1):
                            touched[hh][b] = True
                sts[k] = None

            # pipelined i-loop: pair(k+1) | exp(k) | fillers | AV(k)
            emit_pair(0)
            for k in range(n_i):
                if k + 1 < n_i:
                    emit_pair(k + 1)
                emit_exp(k)
                pump(1)
                emit_av(k)

            # stage row sums (bf16, 1 lane) + o rows (bf16) so o_ps frees
            for hh in range(2):
                h = 2 * p + hh
                rsb = work.tile([1, TJ], BF16, tag="rsb", bufs=4)
                nc.vector.tensor_copy(rsb[:], o_ps[hh][64:65, :])
                nc.sync.dma_start(out=rcat_cur[0][h : h + 1, :], in_=rsb[:])
                nc.vector.tensor_copy(
                    osb_cur[0][p][hh * 64 : (hh + 1) * 64, :], o_ps[hh][0:64, :]
                )

        rcat_cur = [None]
        osb_cur = [None]

        # ---------------------------------------------------------------
        # Prologue
        # ---------------------------------------------------------------
        # warm the ACT exp table while DMAs run
        nc.vector.memset(dummy_sb[:], 0.0)
        nc.scalar.activation(dummy_sb[:, 0:1], dummy_sb[:, 1:2], EXP, scale=1.0)
        for i in range(NSI):
            nc.vector.memset(
                xv_sb[i][:].rearrange("p (h x) -> p h x", x=65)[:, :, 64:65], 1.0
            )

        alloc_stream_tiles(0)
        # first-needed DMAs first: wq+qs(0), then wk+ks(0), wv+vs(0)
        for e in range(EC):
            nc.sync.dma_start(out=wq_sb[e][:], in_=wq[e * 128 : (e + 1) * 128, :])
            dma_stream(qs_j[0], qTt, 0, e)
        for e in range(EC):
            nc.sync.dma_start(out=wk_sb[e][:], in_=wk[e * 128 : (e + 1) * 128, :])
            dma_stream(ks_j[0], kTt, 0, e)
        for e in range(EC):
            nc.sync.dma_start(out=wv_sb[e][:], in_=wv[e * 128 : (e + 1) * 128, :])
            dma_stream(vs_j[0], vTt, 0, e)
        nc.sync.dma_start(out=selbc_sb[:], in_=selbc[:])
        for u in range(n_pat):
            nc.sync.dma_start(out=pat_sb[u][:], in_=pat[u * 128 : (u + 1) * 128, :])
        for p in range(NP):
            nc.sync.dma_start(out=wp_sb[p][:], in_=wpT[p * 128 : (p + 1) * 128, :])
        alloc_stream_tiles(1)
        emit_stream_dmas(1)

        # inline p0 projections for j=0, rest queued
        push_qk_chain(0, 0)
        require(("xk", 0, 0))
        push_proj(0, first_p_inline=True)

        # ---------------------------------------------------------------
        # Main loop
        # ---------------------------------------------------------------
        for j in range(NJ):
            if j + 1 < NJ:
                push_proj(j + 1)

            rcat_cur[0] = work.tile([HPC, TJ], BF16, tag="rcat", bufs=2, name=f"rcat_{j}")
            osb_cur[0] = [
                work.tile([128, TJ], BF16, tag=f"osb{p}", bufs=2, name=f"osb{p}_{j}")
                for p in range(NP)
            ]
            osc_sb = osc_sb_all[j % 2]
            rcat_sb = rcat_cur[0]
            osb_sb = osb_cur[0]

            for p in range(NP):
                if j > 0:
                    require(("osc", p, j - 1))   # frees osb tag before restage
                emit_attention(p, j)

            # all proj(j) closures are drained by now (att(j) p=3 required
            # ("xk", 3, j)), so the buffer-recycling DMA is safe to emit
            if j + 2 < NJ:
                alloc_stream_tiles(j + 2)
                emit_stream_dmas(j + 2)

            push_tail(j, rcat_sb, osb_sb, osc_sb)
            push_y(j, osc_sb)

        # drain everything left (tail + y of the last tiles)
        while fillers:
            _run_one()

    _split_multi_waits(nc)
    return nc


_SELBC = np.zeros((HPC, DC), NPBF16)
for _p in range(HPC // 2):
    _SELBC[2 * _p, _p * 128 : _p * 128 + 64] = 1.0
    _SELBC[2 * _p + 1, _p * 128 + 64 : _p * 128 + 128] = 1.0

_CACHE = {}


def _get_program(mask):
    key = np.asarray(mask, dtype=bool).tobytes()
    prog = _CACHE.get(key)
    if prog is None:
        _install_patches()
        btab, patterns = _classify_mask(mask)
        nc = _build(btab, len(patterns))
        prog = (nc, patterns)
        _CACHE[key] = prog
    return prog


def _prepare(k, q, v, mask, Wk, Wq, Wv, Wp):
    """Build (cached) the SPMD program and the 8 per-core input maps."""
    k = np.asarray(k, np.float32)
    q = np.asarray(q, np.float32)
    v = np.asarray(v, np.float32)
    Wk = np.asarray(Wk, np.float32)
    Wq = np.asarray(Wq, np.float32)
    Wv = np.asarray(Wv, np.float32)
    Wp = np.asarray(Wp, np.float32)

    nc, patterns = _get_program(mask)
    patflat = np.ascontiguousarray(patterns.reshape(-1, 128))

    def tr_tiled(x):  # [T, E] f32 -> [NJ*EC*128, TJ] bf16, tile (j,e) contiguous
        xt = np.ascontiguousarray(x.astype(NPBF16).T)      # [E, T]
        xt = xt.reshape(EC, 128, NJ, TJ).transpose(2, 0, 1, 3)
        return np.ascontiguousarray(xt.reshape(NJ * EC * 128, TJ))

    def wcat(W, half):  # [H, E, D] -> [E, 512] bf16 for this half's 8 heads
        return np.ascontiguousarray(
            W[half * HPC : (half + 1) * HPC].transpose(1, 0, 2).reshape(E, DC)
        ).astype(NPBF16)

    in_maps = []
    for c in range(8):
        b, half = divmod(c, 2)
        off = half * DC
        in_maps.append(
            {
                "qTt": tr_tiled(q[b]),
                "kTt": tr_tiled(k[b]),
                "vTt": tr_tiled(v[b]),
                "wq": wcat(Wq, half),
                "wk": wcat(Wk, half),
                "wv": wcat(Wv, half),
                "wpT": np.ascontiguousarray(Wp[:, off : off + DC].T).astype(NPBF16),
                "pat": patflat,
                "selbc": _SELBC,
            }
        )
    return nc, in_maps


def kernel(k, q, v, mask, Wk, Wq, Wv, Wp, bp):
    bp = np.asarray(bp, np.float32)
    nc, in_maps = _prepare(k, q, v, mask, Wk, Wq, Wv, Wp)
    res = run_bass_kernel_spmd(nc, in_maps, list(range(8)))
    out = np.empty((B, T, E), np.float32)
    for b in range(B):
        yt = res.results[2 * b]["yTt"] + res.results[2 * b + 1]["yTt"]
        # [NJ*EC*128, TJ] -> [E, T]
        yt = yt.reshape(NJ, EC, 128, TJ).transpose(1, 2, 0, 3).reshape(E, T)
        out[b] = yt.T + bp[None, :]
    return out


# revision 11
# speedup vs baseline: 1.0697x; 1.0697x over previous
"""Multi-head causal attention (B=4, T=2048, E=1024, H=16, D=64) on 8 trn2
NeuronCores via Bass/Tile.

Sharding: core c handles batch b = c//2 and heads [half*8, half*8+8), half =
c%2. Each core computes its 8 heads' attention and a partial output
projection; the host sums the two half partials per batch, transposes, and
adds the bias.

On-device layout is "transposed": activations are [feature, token] so every
matmul contracts over the partition dim. Softmax denominators come from a
ones-column appended to the stationary V operand (M=65 matmuls); masking is
applied block-wise (128x128) with patterns derived from the actual mask input
at build time. No max-subtraction is needed: scores are ~N(0, 0.083^2).

This version software-pipelines the whole kernel: the attention i-loop is
ACT(exp)-paced, so projection matmuls for the next t-tile, output-projection
matmuls for the previous t-tile, and softmax-tail work are injected as
"filler" closures between attention steps to keep the PE busy. Inputs are
pre-tiled host-side so every DMA moves one contiguous 128KB block.
"""
import numpy as np
import ml_dtypes
from collections import deque
from contextlib import ExitStack

import concourse.bass as bass
import concourse.mybir as mybir
import concourse.tile as tile
from concourse.bass_utils import run_bass_kernel_spmd
from concourse.vector_clock import ScopedClock

BF16 = mybir.dt.bfloat16
F32 = mybir.dt.float32
NPBF16 = ml_dtypes.bfloat16

B, T, E, H, D = 4, 2048, 1024, 16, 64
HPC = 8            # heads per core
DC = HPC * D       # 512: stacked head dim per core
TJ = 512           # t tile (matmul free dim)
NJ = T // TJ       # 4
SI = 128           # s tile (psum partition dim)
NSI = T // SI      # 16
EC = E // 128      # 8 e-chunks
NP = HPC // 2      # 4 head pairs

# ---------------------------------------------------------------------------
# Workarounds for this walrus build: at most ONE sync wait per instruction.
# ---------------------------------------------------------------------------
_PATCHED = False


def _patched_drain_and_barrier(self, tick_clock, wait_clock):
    drain_inst = self.nc.sync.drain(fusable=False)
    wait_clock.add_sem_waits(
        drain_inst.ins, ScopedClock({None: tick_clock.global_clock})
    )
    si = drain_inst.ins.sync_info
    if si is not None and len(si.on_wait) > 1:
        waits = list(si.on_wait)
        drain_inst.ins.sync_info = mybir.SyncInfo(
            on_wait=waits[:1], on_update=list(si.on_update)
        )
        for ofs in range(1, len(waits)):
            extra = self.nc.sync.drain(fusable=False)
            extra.ins.sync_info = mybir.SyncInfo(
                on_wait=waits[ofs : ofs + 1], on_update=[]
            )
    self.nc.all_engine_barrier()
    assert self.sems is not None
    popped = self.nc._tile_sem_poison_stack.pop()
    assert popped is self._sem_poison
    self.nc.clear_and_free_semaphores(list(self.sems.allocated().values()))
    self.nc.all_engine_barrier()


def _install_patches():
    global _PATCHED
    if _PATCHED:
        return
    tile.TileContext._drain_and_barrier = _patched_drain_and_barrier
    _PATCHED = True


def _make_carrier(nc, engine, wait):
    """Wait-only EventSemaphore on `engine` (cheap: ~70ns, no pipe flush)."""
    ev = mybir.InstEventSemaphore(name=f"W-{nc.next_id()}", ins=[], outs=[])
    ev.engine = engine
    ev.sync_info = mybir.SyncInfo(on_wait=[wait], on_update=[])
    return ev


_ENGINE_SEM = {
    "EngineType.PE": "PE",
    "EngineType.DVE": "DVE",
    "EngineType.Activation": "Activation",
    "EngineType.SP": "SP",
    "EngineType.Pool": "Pool",
}
# engines with in-order issue AND in-order completion for these inst types:
# a wait on the engine's own completion sem is redundant. Ldweights excluded
# (the PE reorder window pulls it ahead of in-flight matmuls).
_DROPPABLE = (
    "InstMatmult", "InstActivation", "InstTensorTensor", "InstTensorCopy",
    "InstTensorReduce", "InstMemset", "InstReciprocal", "InstDMACopy",
    "InstCopyPredicated", "InstTensorScalarPtr", "InstTensorScalar",
    "InstCast", "InstDveOp", "InstCustomDve",
)


def _split_multi_waits(nc):
    for bbw in list(nc.bb_map.values()):
        bb = bbw.bb
        insts = bb.instructions
        if not any(
            getattr(i, "sync_info", None) is not None and len(i.sync_info.on_wait) > 1
            for i in insts
        ):
            continue
        out = []
        for inst in insts:
            si = getattr(inst, "sync_info", None)
            waits = list(si.on_wait) if si is not None else []
            if len(waits) > 1:
                own = _ENGINE_SEM.get(str(inst.engine))
                tn = type(inst).__name__
                if own is not None and tn.startswith(_DROPPABLE):
                    waits = [
                        w for w in waits
                        if w.ant_name.rsplit("_", 1)[0] != own
                    ] or waits[-1:]
            if len(waits) > 1:
                for w in waits[:-1]:
                    out.append(_make_carrier(nc, inst.engine, w))
                waits = waits[-1:]
            if si is not None and list(si.on_wait) != waits:
                inst.sync_info = mybir.SyncInfo(
                    on_wait=waits, on_update=list(si.on_update)
                )
            out.append(inst)
        insts[:] = out


# ---------------------------------------------------------------------------
# Mask analysis (host side, 128x128 blocks).
# ---------------------------------------------------------------------------
def _classify_mask(mask):
    """mask: [T, T] bool, mask[t, s]=True means masked (score -> -inf).

    Returns (btab, patterns): btab[i][jj] in {'skip', 'dense', int u};
    patterns[u] is a [128,128] bf16 multiplier in [s, t] orientation."""
    nb = T // 128
    m = np.asarray(mask, dtype=bool)
    patterns = []
    index = {}
    btab = [[None] * nb for _ in range(nb)]
    for i in range(nb):          # s block
        for jj in range(nb):     # t block
            sub = m[jj * 128 : (jj + 1) * 128, i * 128 : (i + 1) * 128]  # [t, s]
            if sub.all():
                btab[i][jj] = "skip"
            elif not sub.any():
                btab[i][jj] = "dense"
            else:
                pat = (~sub).T.astype(NPBF16)  # [s, t] multiplier
                key = pat.tobytes()
                if key not in index:
                    index[key] = len(patterns)
                    patterns.append(pat)
                btab[i][jj] = index[key]
    if not patterns:
        patterns.append(np.ones((128, 128), NPBF16))
    return btab, np.stack(patterns)


# ---------------------------------------------------------------------------
# Kernel builder (SPMD program, identical on all 8 cores).
# ---------------------------------------------------------------------------
def _build(btab, n_pat):
    nc = bass.Bass()
    # pre-tiled streams: tile (j, e) is rows (j*EC+e)*128 .. +128, contiguous
    qTt = nc.declare_dram_parameter("qTt", [NJ * EC * 128, TJ], BF16, isOutput=False)
    kTt = nc.declare_dram_parameter("kTt", [NJ * EC * 128, TJ], BF16, isOutput=False)
    vTt = nc.declare_dram_parameter("vTt", [NJ * EC * 128, TJ], BF16, isOutput=False)
    wq = nc.declare_dram_parameter("wq", [E, DC], BF16, isOutput=False)
    wk = nc.declare_dram_parameter("wk", [E, DC], BF16, isOutput=False)
    wv = nc.declare_dram_parameter("wv", [E, DC], BF16, isOutput=False)
    wpT = nc.declare_dram_parameter("wpT", [DC, E], BF16, isOutput=False)
    pat = nc.declare_dram_parameter("pat", [n_pat * 128, 128], BF16, isOutput=False)
    selbc = nc.declare_dram_parameter("selbc", [HPC, DC], BF16, isOutput=False)
    # pre-tiled output: tile (j, m) is rows (j*EC+m)*128 .. +128
    yTt = nc.declare_dram_parameter("yTt", [NJ * EC * 128, TJ], F32, isOutput=True)

    EXP = mybir.ActivationFunctionType.Exp
    LN = mybir.ActivationFunctionType.Ln

    with ExitStack() as ctx:
        tc = ctx.enter_context(tile.TileContext(nc))
        # SBUF pools
        consts = ctx.enter_context(tc.tile_pool(name="consts", bufs=1))
        streams = ctx.enter_context(tc.tile_pool(name="streams", bufs=1))
        acts = ctx.enter_context(tc.tile_pool(name="acts", bufs=1))
        work = ctx.enter_context(tc.tile_pool(name="work", bufs=1))
        # PSUM pools
        psA = ctx.enter_context(tc.tile_pool(name="psA", bufs=1, space="PSUM"))
        psB = ctx.enter_context(tc.tile_pool(name="psB", bufs=1, space="PSUM"))

        # ---- constants ----
        wq_sb = [consts.tile([128, DC], BF16, tag=f"wq{e}", name=f"wq{e}", bufs=1) for e in range(EC)]
        wk_sb = [consts.tile([128, DC], BF16, tag=f"wk{e}", name=f"wk{e}", bufs=1) for e in range(EC)]
        wv_sb = [consts.tile([128, DC], BF16, tag=f"wv{e}", name=f"wv{e}", bufs=1) for e in range(EC)]
        wp_sb = [consts.tile([128, E], BF16, tag=f"wp{p}", name=f"wp{p}", bufs=1) for p in range(NP)]
        pat_sb = [consts.tile([128, 128], BF16, tag=f"pat{u}", name=f"pat{u}", bufs=1) for u in range(n_pat)]
        selbc_sb = consts.tile([HPC, DC], BF16, tag="selbc", name="selbc", bufs=1)
        dummy_sb = consts.tile([1, 2], BF16, tag="dummy", name="dummy", bufs=1)

        # ---- persistent activations (per (p, j) tiles so pipelined writes
        # to tile j+1 never alias reads of tile j) ----
        xq_t = [[acts.tile([128, TJ], BF16, tag=f"xq{p}_{j}", name=f"xq{p}_{j}", bufs=1)
                 for j in range(NJ)] for p in range(NP)]
        xk_t = [[acts.tile([128, TJ], BF16, tag=f"xk{p}_{j}", name=f"xk{p}_{j}", bufs=1)
                 for j in range(NJ)] for p in range(NP)]
        # xv tiles: per s-tile, heads laid out as 8 x (64 cols xv | 1 col ones)
        xv_sb = [acts.tile([128, HPC * 65], BF16, tag=f"xv{i}", name=f"xv{i}", bufs=1) for i in range(NSI)]
        osc_sb_all = [
            [acts.tile([128, TJ], BF16, tag=f"osc{p}_{jj}", name=f"osc{p}_{jj}", bufs=1)
             for p in range(NP)]
            for jj in range(2)
        ]

        # ---------------------------------------------------------------
        # Filler queue: closures emitting ~1-2 PE matmuls (+ their DVE/DMA
        # tails). `require(key)` force-drains through a named closure.
        # ---------------------------------------------------------------
        fillers = deque()        # entries: (cost, fn)
        ready_idx = {}           # key -> push counter of last closure for key
        drained = [0]            # count of executed closures
        pushed = [0]
        debt = [0.0]

        def push(cost, fn, key=None):
            fillers.append((cost, fn))
            pushed[0] += 1
            if key is not None:
                ready_idx[key] = pushed[0]

        def _run_one():
            cost, fn = fillers.popleft()
            fn()
            drained[0] += 1
            return cost

        def pump(units):
            debt[0] += units
            while fillers and debt[0] >= fillers[0][0]:
                debt[0] -= _run_one()

        def require(key):
            idx = ready_idx.get(key, 0)
            while drained[0] < idx:
                _run_one()

        # ---------------------------------------------------------------
        # DMA emission
        # ---------------------------------------------------------------
        def dma_stream(dst_pool_tiles, src, j, e):
            r0 = (j * EC + e) * 128
            nc.sync.dma_start(out=dst_pool_tiles[e][:], in_=src[r0 : r0 + 128, :])

        def emit_stream_dmas(j):
            for e in range(EC):
                dma_stream(qs_j[j], qTt, j, e)
                dma_stream(ks_j[j], kTt, j, e)
                dma_stream(vs_j[j], vTt, j, e)

        # explicit per-j stream tile handles (2 buffer sets, alternating)
        qs_j, ks_j, vs_j = {}, {}, {}

        def alloc_stream_tiles(j):
            qs_j[j] = [streams.tile([128, TJ], BF16, tag=f"qs{e}", name=f"qs{e}_{j}", bufs=2) for e in range(EC)]
            ks_j[j] = [streams.tile([128, TJ], BF16, tag=f"ks{e}", name=f"ks{e}_{j}", bufs=2) for e in range(EC)]
            vs_j[j] = [streams.tile([128, TJ], BF16, tag=f"vs{e}", name=f"vs{e}_{j}", bufs=2) for e in range(EC)]

        # ---------------------------------------------------------------
        # Projection chain closures
        # ---------------------------------------------------------------
        def push_qk_chain(p, j):
            """xq and xk chains for (p, j): 4 closures of 4 matmuls."""
            pc = slice(p * 128, (p + 1) * 128)
            st_q = {}

            def q1():
                ps = psA.tile([128, TJ], F32, tag="mm512", bufs=2, name=f"xqp{p}_{j}")
                st_q['ps'] = ps
                for e in range(4):
                    nc.tensor.matmul(ps[:], wq_sb[e][:, pc], qs_j[j][e][:],
                                     start=(e == 0), stop=False)

            def q2():
                ps = st_q['ps']
                for e in range(4, EC):
                    nc.tensor.matmul(ps[:], wq_sb[e][:, pc], qs_j[j][e][:],
                                     start=False, stop=(e == EC - 1))
                nc.vector.tensor_copy(xq_t[p][j][:], ps[:])

            st_k = {}

            def k1():
                ps = psA.tile([128, TJ], F32, tag="mm512", bufs=2, name=f"xkp{p}_{j}")
                st_k['ps'] = ps
                for e in range(4):
                    nc.tensor.matmul(ps[:], wk_sb[e][:, pc], ks_j[j][e][:],
                                     start=(e == 0), stop=False)

            def k2():
                ps = st_k['ps']
                for e in range(4, EC):
                    nc.tensor.matmul(ps[:], wk_sb[e][:, pc], ks_j[j][e][:],
                                     start=False, stop=(e == EC - 1))
                nc.vector.tensor_copy(xk_t[p][j][:], ps[:])

            push(2, q1)
            push(2, q2, key=("xq", p, j))
            push(2, k1)
            push(2, k2, key=("xk", p, j))

        def push_xv_chain(loc, j):
            si = 4 * j + loc
            st = {}

            def v1():
                ps = psA.tile([128, DC], F32, tag="mm512", bufs=2, name=f"xvp{si}")
                st['ps'] = ps
                for e in range(4):
                    nc.tensor.matmul(ps[:], vs_j[j][e][:, loc * 128 : (loc + 1) * 128],
                                     wv_sb[e][:], start=(e == 0), stop=False)

            def v2():
                ps = st['ps']
                for e in range(4, EC):
                    nc.tensor.matmul(ps[:], vs_j[j][e][:, loc * 128 : (loc + 1) * 128],
                                     wv_sb[e][:], start=False, stop=(e == EC - 1))
                nc.vector.tensor_copy(
                    xv_sb[si][:].rearrange("p (h x) -> p h x", x=65)[:, :, 0:64],
                    ps[:].rearrange("p (h d) -> p h d", h=HPC),
                )

            push(2, v1)
            push(2, v2, key=("xv", si))

        def push_proj(j, first_p_inline=False):
            """All projection chains for t-tile j, p0's q/k first."""
            order = []
            if not first_p_inline:
                order.append(("qk", 0))
            order += [("xv", loc) for loc in range(4)]
            order += [("qk", p) for p in range(1, NP)]
            for kind, a in order:
                if kind == "qk":
                    push_qk_chain(a, j)
                else:
                    push_xv_chain(a, j)

        # ---------------------------------------------------------------
        # Softmax tail + output projection closures
        # ---------------------------------------------------------------
        def push_tail(j, rcat_sb, osb_sb, osc_sb):
            # 1/r on ACT via exp(-ln r): exp+ln share one table set, and the
            # DVE reciprocal is lane-starved on [8, TJ] (3.3us); ACT does the
            # pair in ~1.4us off the DVE critical path.
            rln = work.tile([HPC, TJ], F32, tag="rrcat32", bufs=2, name=f"rln_{j}")
            nc.scalar.activation(rln[:], rcat_sb[:], LN, scale=1.0)
            rr = work.tile([HPC, TJ], BF16, tag="rrcat", bufs=2, name=f"rrc_{j}")
            nc.scalar.activation(rr[:], rln[:], EXP, scale=-1.0)
            tail_state[j] = rr

            for p in range(NP):
                def mk(p):
                    def rbosc():
                        rr = tail_state[j]
                        rb_ps = psA.tile([128, TJ], F32, tag="mm512", bufs=2,
                                         name=f"rb_{p}_{j}")
                        nc.tensor.matmul(
                            rb_ps[:], selbc_sb[:, p * 128 : (p + 1) * 128], rr[:],
                            start=True, stop=True,
                        )
                        nc.vector.tensor_mul(osc_sb[p][:], osb_sb[p][:], rb_ps[:])
                    return rbosc
                push(1, mk(p), key=("osc", p, j))

        def push_y(j, osc_sb):
            for m in range(EC):
                def mk(m):
                    def ychunk():
                        y_ps = psA.tile([128, TJ], F32, tag="mm512", bufs=2,
                                        name=f"y_{m}_{j}")
                        for p in range(NP):
                            nc.tensor.matmul(
                                y_ps[:], wp_sb[p][:, m * 128 : (m + 1) * 128],
                                osc_sb[p][:],
                                start=(p == 0), stop=(p == NP - 1),
                            )
                        y_sb = work.tile([128, TJ], F32, tag="y", bufs=2,
                                         name=f"ysb_{m}_{j}")
                        nc.vector.tensor_copy(y_sb[:], y_ps[:])
                        r0 = (j * EC + m) * 128
                        nc.sync.dma_start(out=yTt[r0 : r0 + 128, :], in_=y_sb[:])
                    return ychunk
                push(2, mk(m))

        tail_state = {}

        # ---------------------------------------------------------------
        # Attention cell for (p, j): ACT-paced i-loop with filler pumping
        # ---------------------------------------------------------------
        def emit_attention(p, j):
            jt = slice(j * TJ, (j + 1) * TJ)
            ivals = []
            for i in range(NSI):
                types = [btab[i][4 * j + bl] for bl in range(4)]
                if all(t == "skip" for t in types):
                    continue
                ivals.append((i, types))
            n_i = len(ivals)

            require(("xq", p, j))

            o_ps = [
                psB.tile([65, TJ], F32, tag=f"ops{hh}", name=f"ops{hh}_{p}_{j}", bufs=1)
                for hh in range(2)
            ]
            touched = [[False] * 4, [False] * 4]
            sts = [None] * n_i     # (st_tile, u_tile, c0)

            def emit_pair(k):
                i, types = ivals[k]
                c0 = next(bl for bl in range(4) if types[bl] != "skip")
                require(("xk", p, i // 4))
                st = psA.tile([128, 2 * TJ], F32, tag="st", bufs=2)
                for hh in range(2):
                    hr = slice(hh * 64, (hh + 1) * 64)
                    nc.tensor.matmul(
                        st[:, hh * TJ + c0 * 128 : (hh + 1) * TJ],
                        xk_t[p][i // 4][hr, (i % 4) * 128 : (i % 4 + 1) * 128],
                        xq_t[p][j][hr, c0 * 128 : TJ],
                        start=True, stop=True,
                    )
                sts[k] = (st, None, c0)

            def emit_exp(k):
                st, _, c0 = sts[k]
                u = work.tile([128, 2 * TJ], BF16, tag="u", bufs=4)
                nc.scalar.activation(
                    u[:].rearrange("p (g c) -> p g c", g=2)[:, :, c0 * 128 : TJ],
                    st[:].rearrange("p (g c) -> p g c", g=2)[:, :, c0 * 128 : TJ],
                    EXP, scale=1.0 / 32.0,
                )
                sts[k] = (st, u, c0)

            def emit_av(k):
                i, types = ivals[k]
                _, u, c0 = sts[k]
                require(("xv", i))
                for hh in range(2):
                    h = 2 * p + hh
                    uo = hh * TJ
                    runs = []  # (bl0, bl1, src_ap)
                    bl = c0
                    while bl < 4:
                        if types[bl] == "dense":
                            b2 = bl
                            while b2 + 1 < 4 and types[b2 + 1] == "dense":
                                b2 += 1
                            runs.append((bl, b2 + 1,
                                         u[:, uo + bl * 128 : uo + (b2 + 1) * 128]))
                            bl = b2 + 1
                        elif types[bl] == "skip":
                            bl += 1
                        else:
                            mt = work.tile([128, 128], BF16, tag="mfix", bufs=4)
                            nc.vector.tensor_mul(
                                mt[:], u[:, uo + bl * 128 : uo + (bl + 1) * 128],
                                pat_sb[types[bl]][:],
                            )
                            runs.append((bl, bl + 1, mt[:]))
                            bl += 1
                    lhs_v = xv_sb[i][:, h * 65 : h * 65 + 65]
                    for ri, (b0, b1, src) in enumerate(runs):
                        first = all(not touched[hh][b] for b in range(b0, b1))
                        assert first == any(
                            not touched[hh][b] for b in range(b0, b1)
                        ), "mask blocks: mixed touch state inside a run"
                        last = (k == n_i - 1) and (ri == len(runs) - 1)
                        nc.tensor.matmul(
                            o_ps[hh][:, b0 * 128 : b1 * 128],
                            lhs_v, src,
                            start=first, stop=last,
                            skip_group_check=True,
                        )
                        for b in range(b0, b1):
                            touched[hh][b] = True
                sts[k] = None

            # pipelined i-loop, two steps per "mode era" to halve the
            # 64<->128 row-tiling mode switches: [pair pair] [exp exp]
            # [fillers AV AV]
            emit_pair(0)
            if n_i > 1:
                emit_pair(1)
            emit_exp(0)
            if n_i > 1:
                emit_exp(1)
            k = 0
            while k < n_i:
                k2 = min(k + 2, n_i)
                for kk in range(k + 2, min(k + 4, n_i)):
                    emit_pair(kk)
                for kk in range(k + 2, min(k + 4, n_i)):
                    emit_exp(kk)
                pump(k2 - k)
                for kk in range(k, k2):
                    emit_av(kk)
                k = k2

            require(("osc", p, j - 1))  # tail(j-1)'s osc mul, before osb restage
            # stage row sums (bf16, 1 lane) + o rows (bf16) so o_ps frees
            for hh in range(2):
                h = 2 * p + hh
                rsb = work.tile([1, TJ], BF16, tag="rsb", bufs=4)
                nc.vector.tensor_copy(rsb[:], o_ps[hh][64:65, :])
                nc.sync.dma_start(out=rcat_cur[0][h : h + 1, :], in_=rsb[:])
                nc.vector.tensor_copy(
                    osb_cur[0][p][hh * 64 : (hh + 1) * 64, :], o_ps[hh][0:64, :]
                )

        rcat_cur = [None]
        osb_cur = [None]

        # ---------------------------------------------------------------
        # Prologue
        # ---------------------------------------------------------------
        # warm the ACT exp table while DMAs run
        nc.vector.memset(dummy_sb[:], 0.0)
        nc.scalar.activation(dummy_sb[:, 0:1], dummy_sb[:, 1:2], EXP, scale=1.0)
        for i in range(NSI):
            nc.vector.memset(
                xv_sb[i][:].rearrange("p (h x) -> p h x", x=65)[:, :, 64:65], 1.0
            )

        alloc_stream_tiles(0)
        # first-needed DMAs first: wq+qs(0), then wk+ks(0), wv+vs(0)
        for e in range(EC):
            nc.sync.dma_start(out=wq_sb[e][:], in_=wq[e * 128 : (e + 1) * 128, :])
            dma_stream(qs_j[0], qTt, 0, e)
        for e in range(EC):
            nc.sync.dma_start(out=wk_sb[e][:], in_=wk[e * 128 : (e + 1) * 128, :])
            dma_stream(ks_j[0], kTt, 0, e)
        for e in range(EC):
            nc.sync.dma_start(out=wv_sb[e][:], in_=wv[e * 128 : (e + 1) * 128, :])
            dma_stream(vs_j[0], vTt, 0, e)
        nc.sync.dma_start(out=selbc_sb[:], in_=selbc[:])
        for u in range(n_pat):
            nc.sync.dma_start(out=pat_sb[u][:], in_=pat[u * 128 : (u + 1) * 128, :])
        for p in range(NP):
            nc.sync.dma_start(out=wp_sb[p][:], in_=wpT[p * 128 : (p + 1) * 128, :])
        alloc_stream_tiles(1)
        emit_stream_dmas(1)

        # inline p0 projections for j=0, rest queued
        push_qk_chain(0, 0)
        require(("xk", 0, 0))
        push_proj(0, first_p_inline=True)

        # ---------------------------------------------------------------
        # Main loop
        # ---------------------------------------------------------------
        for j in range(NJ):
            if j + 1 < NJ:
                push_proj(j + 1)

            rcat_cur[0] = work.tile([HPC, TJ], BF16, tag="rcat", bufs=2, name=f"rcat_{j}")
            osb_cur[0] = [
                work.tile([128, TJ], BF16, tag=f"osb{p}", bufs=2, name=f"osb{p}_{j}")
                for p in range(NP)
            ]
            osc_sb = osc_sb_all[j % 2]
            rcat_sb = rcat_cur[0]
            osb_sb = osb_cur[0]

            for p in range(NP):
                emit_attention(p, j)

            # all proj(j) closures are drained by now (att(j) p=3 required
            # ("xk", 3, j)), so the buffer-recycling DMA is safe to emit
            if j + 2 < NJ:
                alloc_stream_tiles(j + 2)
                emit_stream_dmas(j + 2)

            push_tail(j, rcat_sb, osb_sb, osc_sb)
            push_y(j, osc_sb)

        # drain everything left (tail + y of the last tiles)
        while fillers:
            _run_one()

    _split_multi_waits(nc)
    return nc


_SELBC = np.zeros((HPC, DC), NPBF16)
for _p in range(HPC // 2):
    _SELBC[2 * _p, _p * 128 : _p * 128 + 64] = 1.0
    _SELBC[2 * _p + 1, _p * 128 + 64 : _p * 128 + 128] = 1.0

_CACHE = {}


def _get_program(mask):
    key = np.asarray(mask, dtype=bool).tobytes()
    prog = _CACHE.get(key)
    if prog is None:
        _install_patches()
        btab, patterns = _classify_mask(mask)
        nc = _build(btab, len(patterns))
        prog = (nc, patterns)
        _CACHE[key] = prog
    return prog


def _prepare(k, q, v, mask, Wk, Wq, Wv, Wp):
    """Build (cached) the SPMD program and the 8 per-core input maps."""
    k = np.asarray(k, np.float32)
    q = np.asarray(q, np.float32)
    v = np.asarray(v, np.float32)
    Wk = np.asarray(Wk, np.float32)
    Wq = np.asarray(Wq, np.float32)
    Wv = np.asarray(Wv, np.float32)
    Wp = np.asarray(Wp, np.float32)

    nc, patterns = _get_program(mask)
    patflat = np.ascontiguousarray(patterns.reshape(-1, 128))

    def tr_tiled(x):  # [T, E] f32 -> [NJ*EC*128, TJ] bf16, tile (j,e) contiguous
        xt = np.ascontiguousarray(x.astype(NPBF16).T)      # [E, T]
        xt = xt.reshape(EC, 128, NJ, TJ).transpose(2, 0, 1, 3)
        return np.ascontiguousarray(xt.reshape(NJ * EC * 128, TJ))

    def wcat(W, half):  # [H, E, D] -> [E, 512] bf16 for this half's 8 heads
        return np.ascontiguousarray(
            W[half * HPC : (half + 1) * HPC].transpose(1, 0, 2).reshape(E, DC)
        ).astype(NPBF16)

    in_maps = []
    for c in range(8):
        b, half = divmod(c, 2)
        off = half * DC
        in_maps.append(
            {
                "qTt": tr_tiled(q[b]),
                "kTt": tr_tiled(k[b]),
                "vTt": tr_tiled(v[b]),
                "wq": wcat(Wq, half),
                "wk": wcat(Wk, half),
                "wv": wcat(Wv, half),
                "wpT": np.ascontiguousarray(Wp[:, off : off + DC].T).astype(NPBF16),
                "pat": patflat,
                "selbc": _SELBC,
            }
        )
    return nc, in_maps


def kernel(k, q, v, mask, Wk, Wq, Wv, Wp, bp):
    bp = np.asarray(bp, np.float32)
    nc, in_maps = _prepare(k, q, v, mask, Wk, Wq, Wv, Wp)
    res = run_bass_kernel_spmd(nc, in_maps, list(range(8)))
    out = np.empty((B, T, E), np.float32)
    for b in range(B):
        yt = res.results[2 * b]["yTt"] + res.results[2 * b + 1]["yTt"]
        # [NJ*EC*128, TJ] -> [E, T]
        yt = yt.reshape(NJ, EC, 128, TJ).transpose(1, 2, 0, 3).reshape(E, T)
        out[b] = yt.T + bp[None, :]
    return out


# revision 23
# speedup vs baseline: 1.1944x; 1.1166x over previous
"""Multi-head causal attention (B=4, T=2048, E=1024, H=16, D=64) on 8 trn2
NeuronCores via Bass/Tile.

Sharding: core c handles batch b = c//2 and heads [half*8, half*8+8), half =
c%2. Each core computes its 8 heads' attention and a partial output
projection; the host sums the two half partials per batch, transposes, and
adds the bias.

On-device layout is "transposed": activations are [feature, token] so every
matmul contracts over the partition dim. Softmax denominators come from a
ones-column appended to the stationary V operand (M=65 matmuls); masking is
applied block-wise (128x128) with patterns derived from the actual mask input
at build time. No max-subtraction is needed: scores are ~N(0, 0.083^2).

This version software-pipelines the whole kernel: the attention i-loop is
ACT(exp)-paced, so projection matmuls for the next t-tile, output-projection
matmuls for the previous t-tile, and softmax-tail work are injected as
"filler" closures between attention steps to keep the PE busy. Inputs are
pre-tiled host-side so every DMA moves one contiguous 128KB block.
"""
import numpy as np
import ml_dtypes
from collections import deque
from contextlib import ExitStack

import concourse.bass as bass
import concourse.mybir as mybir
import concourse.tile as tile
from concourse.bass_utils import run_bass_kernel_spmd
from concourse.vector_clock import ScopedClock

BF16 = mybir.dt.bfloat16
F32 = mybir.dt.float32
NPBF16 = ml_dtypes.bfloat16
NPFP8 = ml_dtypes.float8_e4m3fn

B, T, E, H, D = 4, 2048, 1024, 16, 64
HPC = 8            # heads per core
DC = HPC * D       # 512: stacked head dim per core
TJ = 512           # t tile (matmul free dim)
NJ = T // TJ       # 4
SI = 128           # s tile (psum partition dim)
NSI = T // SI      # 16
EC = E // 128      # 8 e-chunks
NP = HPC // 2      # 4 head pairs

# ---------------------------------------------------------------------------
# Workarounds for this walrus build: at most ONE sync wait per instruction.
# ---------------------------------------------------------------------------
_PATCHED = False


def _patched_drain_and_barrier(self, tick_clock, wait_clock):
    drain_inst = self.nc.sync.drain(fusable=False)
    wait_clock.add_sem_waits(
        drain_inst.ins, ScopedClock({None: tick_clock.global_clock})
    )
    si = drain_inst.ins.sync_info
    if si is not None and len(si.on_wait) > 1:
        waits = list(si.on_wait)
        drain_inst.ins.sync_info = mybir.SyncInfo(
            on_wait=waits[:1], on_update=list(si.on_update)
        )
        for ofs in range(1, len(waits)):
            extra = self.nc.sync.drain(fusable=False)
            extra.ins.sync_info = mybir.SyncInfo(
                on_wait=waits[ofs : ofs + 1], on_update=[]
            )
    self.nc.all_engine_barrier()
    assert self.sems is not None
    popped = self.nc._tile_sem_poison_stack.pop()
    assert popped is self._sem_poison
    self.nc.clear_and_free_semaphores(list(self.sems.allocated().values()))
    self.nc.all_engine_barrier()


def _install_patches():
    global _PATCHED
    if _PATCHED:
        return
    tile.TileContext._drain_and_barrier = _patched_drain_and_barrier
    _PATCHED = True


def _make_carrier(nc, engine, wait):
    """Wait-only EventSemaphore on `engine` (cheap: ~70ns, no pipe flush)."""
    ev = mybir.InstEventSemaphore(name=f"W-{nc.next_id()}", ins=[], outs=[])
    ev.engine = engine
    ev.sync_info = mybir.SyncInfo(on_wait=[wait], on_update=[])
    return ev


_ENGINE_SEM = {
    "EngineType.PE": "PE",
    "EngineType.DVE": "DVE",
    "EngineType.Activation": "Activation",
    "EngineType.SP": "SP",
    "EngineType.Pool": "Pool",
}
# engines with in-order issue AND in-order completion for these inst types:
# a wait on the engine's own completion sem is redundant. Ldweights excluded
# (the PE reorder window pulls it ahead of in-flight matmuls).
_DROPPABLE = (
    "InstMatmult", "InstActivation", "InstTensorTensor", "InstTensorCopy",
    "InstTensorReduce", "InstMemset", "InstReciprocal", "InstDMACopy",
    "InstCopyPredicated", "InstTensorScalarPtr", "InstTensorScalar",
    "InstCast", "InstDveOp", "InstCustomDve",
)


def _split_multi_waits(nc):
    for bbw in list(nc.bb_map.values()):
        bb = bbw.bb
        insts = bb.instructions
        if not any(
            getattr(i, "sync_info", None) is not None and len(i.sync_info.on_wait) > 1
            for i in insts
        ):
            continue
        out = []
        for inst in insts:
            si = getattr(inst, "sync_info", None)
            waits = list(si.on_wait) if si is not None else []
            if len(waits) > 1:
                own = _ENGINE_SEM.get(str(inst.engine))
                tn = type(inst).__name__
                if own is not None and tn.startswith(_DROPPABLE):
                    waits = [
                        w for w in waits
                        if w.ant_name.rsplit("_", 1)[0] != own
                    ] or waits[-1:]
            if len(waits) > 1:
                for w in waits[:-1]:
                    out.append(_make_carrier(nc, inst.engine, w))
                waits = waits[-1:]
            if si is not None and list(si.on_wait) != waits:
                inst.sync_info = mybir.SyncInfo(
                    on_wait=waits, on_update=list(si.on_update)
                )
            out.append(inst)
        insts[:] = out


# ---------------------------------------------------------------------------
# Mask analysis (host side, 128x128 blocks).
# ---------------------------------------------------------------------------
def _classify_mask(mask):
    """mask: [T, T] bool, mask[t, s]=True means masked (score -> -inf).

    Returns (btab, patterns): btab[i][jj] in {'skip', 'dense', int u};
    patterns[u] is a [128,128] bf16 multiplier in [s, t] orientation."""
    nb = T // 128
    m = np.asarray(mask, dtype=bool)
    patterns = []
    index = {}
    btab = [[None] * nb for _ in range(nb)]
    for i in range(nb):          # s block
        for jj in range(nb):     # t block
            sub = m[jj * 128 : (jj + 1) * 128, i * 128 : (i + 1) * 128]  # [t, s]
            if sub.all():
                btab[i][jj] = "skip"
            elif not sub.any():
                btab[i][jj] = "dense"
            else:
                pat = (~sub).T.astype(NPBF16)  # [s, t] multiplier
                key = pat.tobytes()
                if key not in index:
                    index[key] = len(patterns)
                    patterns.append(pat)
                btab[i][jj] = index[key]
    if not patterns:
        patterns.append(np.ones((128, 128), NPBF16))
    return btab, np.stack(patterns)


# ---------------------------------------------------------------------------
# Kernel builder (SPMD program, identical on all 8 cores).
# ---------------------------------------------------------------------------
def _build(btab, n_pat):
    nc = bass.Bass()
    FP8 = mybir.dt.float8e4
    # pre-tiled streams: tile (j, e) is rows (j*EC+e)*128 .. +128, contiguous
    # q/k are fp8 (DoubleRow matmuls; errors attenuate through softmax)
    qTt = nc.declare_dram_parameter("qTt", [NJ * EC * 128, TJ], FP8, isOutput=False)
    kTt = nc.declare_dram_parameter("kTt", [NJ * EC * 128, TJ], FP8, isOutput=False)
    vTt = nc.declare_dram_parameter("vTt", [NJ * EC * 128, TJ], BF16, isOutput=False)
    wq = nc.declare_dram_parameter("wq", [E, DC], FP8, isOutput=False)
    wk = nc.declare_dram_parameter("wk", [E, DC], FP8, isOutput=False)
    wv = nc.declare_dram_parameter("wv", [E, DC], BF16, isOutput=False)
    wpT = nc.declare_dram_parameter("wpT", [DC, E], BF16, isOutput=False)
    pat = nc.declare_dram_parameter("pat", [n_pat * 128, 128], BF16, isOutput=False)
    selbc = nc.declare_dram_parameter("selbc", [HPC, DC], BF16, isOutput=False)
    # pre-tiled output: tile (j, m) is rows (j*EC+m)*128 .. +128
    yTt = nc.declare_dram_parameter("yTt", [NJ * EC * 128, TJ], F32, isOutput=True)

    EXP = mybir.ActivationFunctionType.Exp
    LN = mybir.ActivationFunctionType.Ln

    with ExitStack() as ctx:
        tc = ctx.enter_context(tile.TileContext(nc))
        # SBUF pools
        consts = ctx.enter_context(tc.tile_pool(name="consts", bufs=1))
        streams = ctx.enter_context(tc.tile_pool(name="streams", bufs=1))
        acts = ctx.enter_context(tc.tile_pool(name="acts", bufs=1))
        work = ctx.enter_context(tc.tile_pool(name="work", bufs=1))
        # PSUM pools
        psA = ctx.enter_context(tc.tile_pool(name="psA", bufs=1, space="PSUM"))
        psB = ctx.enter_context(tc.tile_pool(name="psB", bufs=1, space="PSUM"))

        # ---- constants ----
        # wq/wk as fp8 chunk-PAIR tiles [128, 2, DC] for DoubleRow matmuls
        wq_sb = [consts.tile([128, 2 * DC], FP8, tag=f"wq{c}", name=f"wq{c}", bufs=1) for c in range(EC // 2)]
        wk_sb = [consts.tile([128, 2 * DC], FP8, tag=f"wk{c}", name=f"wk{c}", bufs=1) for c in range(EC // 2)]
        wv_sb = [consts.tile([128, DC], BF16, tag=f"wv{e}", name=f"wv{e}", bufs=1) for e in range(EC)]
        wp_sb = [consts.tile([128, E], BF16, tag=f"wp{p}", name=f"wp{p}", bufs=1) for p in range(NP)]
        pat_sb = [consts.tile([128, 128], BF16, tag=f"pat{u}", name=f"pat{u}", bufs=1) for u in range(n_pat)]
        selbc_sb = consts.tile([HPC, DC], BF16, tag="selbc", name="selbc", bufs=1)
        dummy_sb = consts.tile([1, 2], BF16, tag="dummy", name="dummy", bufs=1)

        # ---- persistent activations (per (p, j) tiles so pipelined writes
        # to tile j+1 never alias reads of tile j) ----
        xq_t = [[acts.tile([128, TJ], BF16, tag=f"xq{p}_{j}", name=f"xq{p}_{j}", bufs=1)
                 for j in range(NJ)] for p in range(NP)]
        xk_t = [[acts.tile([128, TJ], BF16, tag=f"xk{p}_{j}", name=f"xk{p}_{j}", bufs=1)
                 for j in range(NJ)] for p in range(NP)]
        # xv tiles: per s-tile, heads laid out as 8 x (64 cols xv | 1 col ones)
        xv_sb = [acts.tile([128, HPC * 65], BF16, tag=f"xv{i}", name=f"xv{i}", bufs=1) for i in range(NSI)]
        osc_sb_all = [
            [acts.tile([128, TJ], BF16, tag=f"osc{p}_{jj}", name=f"osc{p}_{jj}", bufs=1)
             for p in range(NP)]
            for jj in range(2)
        ]

        # ---------------------------------------------------------------
        # Filler queue: closures emitting ~1-2 PE matmuls (+ their DVE/DMA
        # tails). `require(key)` force-drains through a named closure.
        # ---------------------------------------------------------------
        fillers = deque()        # entries: (cost, fn)
        ready_idx = {}           # key -> push counter of last closure for key
        drained = [0]            # count of executed closures
        pushed = [0]
        debt = [0.0]

        def push(cost, fn, key=None):
            fillers.append((cost, fn))
            pushed[0] += 1
            if key is not None:
                ready_idx[key] = pushed[0]

        def _run_one():
            cost, fn = fillers.popleft()
            fn()
            drained[0] += 1
            return cost

        def pump(units):
            debt[0] += units
            while fillers and debt[0] >= fillers[0][0]:
                debt[0] -= _run_one()

        def require(key):
            idx = ready_idx.get(key, 0)
            while drained[0] < idx:
                _run_one()

        # ---------------------------------------------------------------
        # DMA emission
        # ---------------------------------------------------------------
        def dma_pair(dst_tile, src, j, c, width):
            """two e-chunks (2c, 2c+1) side by side into one pair tile"""
            for sub in range(2):
                r0 = (j * EC + 2 * c + sub) * 128
                nc.sync.dma_start(out=dst_tile[:, sub * width : (sub + 1) * width],
                                  in_=src[r0 : r0 + 128, :])

        def emit_stream_dmas(j):
            for c in range(EC // 2):
                dma_pair(qs_j[j][c], qTt, j, c, TJ)
                dma_pair(ks_j[j][c], kTt, j, c, TJ)
            for e in range(EC):
                r0 = (j * EC + e) * 128
                nc.sync.dma_start(out=vs_j[j][e][:], in_=vTt[r0 : r0 + 128, :])

        # explicit per-j stream tile handles (2 buffer sets, alternating);
        # q/k as fp8 chunk-pair tiles, v as bf16 per-chunk tiles
        qs_j, ks_j, vs_j = {}, {}, {}

        def alloc_stream_tiles(j):
            qs_j[j] = [streams.tile([128, 2 * TJ], FP8, tag=f"qs{c}", name=f"qs{c}_{j}", bufs=2) for c in range(EC // 2)]
            ks_j[j] = [streams.tile([128, 2 * TJ], FP8, tag=f"ks{c}", name=f"ks{c}_{j}", bufs=2) for c in range(EC // 2)]
            vs_j[j] = [streams.tile([128, TJ], BF16, tag=f"vs{e}", name=f"vs{e}_{j}", bufs=2) for e in range(EC)]

        # ---------------------------------------------------------------
        # Projection chain closures
        # ---------------------------------------------------------------
        DR = mybir.MatmulPerfMode.DoubleRow

        def _dr_mm(ps, w_sb, s_tiles, p, c, start, stop):
            pc = slice(p * 128, (p + 1) * 128)
            lhsT = w_sb[c][:].rearrange("r (k m) -> r k m", k=2)[:, :, pc]
            rhs = s_tiles[c][:].rearrange("r (k n) -> r k n", k=2)
            nc.tensor.matmul(ps[:], lhsT, rhs, start=start, stop=stop,
                             perf_mode=DR)

        def push_qk_chain(p, j):
            """xq and xk chains for (p, j): DoubleRow fp8, 2 closures each."""
            st_q = {}

            def q1():
                ps = psA.tile([128, TJ], F32, tag="mm512", bufs=2, name=f"xqp{p}_{j}")
                st_q['ps'] = ps
                for c in range(2):
                    _dr_mm(ps, wq_sb, qs_j[j], p, c, c == 0, False)

            def q2():
                ps = st_q['ps']
                for c in range(2, EC // 2):
                    _dr_mm(ps, wq_sb, qs_j[j], p, c, False, c == EC // 2 - 1)
                nc.vector.tensor_copy(xq_t[p][j][:], ps[:])

            st_k = {}

            def k1():
                ps = psA.tile([128, TJ], F32, tag="mm512", bufs=2, name=f"xkp{p}_{j}")
                st_k['ps'] = ps
                for c in range(2):
                    _dr_mm(ps, wk_sb, ks_j[j], p, c, c == 0, False)

            def k2():
                ps = st_k['ps']
                for c in range(2, EC // 2):
                    _dr_mm(ps, wk_sb, ks_j[j], p, c, False, c == EC // 2 - 1)
                nc.vector.tensor_copy(xk_t[p][j][:], ps[:])

            push(1, q1)
            push(1, q2, key=("xq", p, j))
            push(1, k1)
            push(1, k2, key=("xk", p, j))

        def push_xv_chain(loc, j):
            si = 4 * j + loc
            st = {}

            def v1():
                ps = psA.tile([128, DC], F32, tag="mm512", bufs=2, name=f"xvp{si}")
                st['ps'] = ps
                for e in range(4):
                    nc.tensor.matmul(ps[:], vs_j[j][e][:, loc * 128 : (loc + 1) * 128],
                                     wv_sb[e][:], start=(e == 0), stop=False)

            def v2():
                ps = st['ps']
                for e in range(4, EC):
                    nc.tensor.matmul(ps[:], vs_j[j][e][:, loc * 128 : (loc + 1) * 128],
                                     wv_sb[e][:], start=False, stop=(e == EC - 1))
                nc.vector.tensor_copy(
                    xv_sb[si][:].rearrange("p (h x) -> p h x", x=65)[:, :, 0:64],
                    ps[:].rearrange("p (h d) -> p h d", h=HPC),
                )

            push(2, v1)
            push(2, v2, key=("xv", si))

        def push_proj(j, first_p_inline=False):
            """All projection chains for t-tile j, p0's q/k first."""
            order = []
            if not first_p_inline:
                order.append(("qk", 0))
            order += [("xv", loc) for loc in range(4)]
            order += [("qk", p) for p in range(1, NP)]
            for kind, a in order:
                if kind == "qk":
                    push_qk_chain(a, j)
                else:
                    push_xv_chain(a, j)

        # ---------------------------------------------------------------
        # Softmax tail + output projection closures
        # ---------------------------------------------------------------
        pending_tail = []    # (cost, fn, key) released at p==1 of next att(j)

        def pend(cost, fn, key=None):
            pending_tail.append((cost, fn, key))

        def release_pending():
            for cost, fn, key in pending_tail:
                push(cost, fn, key=key)
            pending_tail.clear()

        def push_tail(j, rcat_sb, osb_sb, osc_sb):
            def recip():
                # 1/r as exp(-ln r) on ACT: exp+ln share one table set; the
                # DVE reciprocal is lane-starved on [8, TJ] (3.3us).
                rln = work.tile([HPC, TJ], F32, tag="rrcat32", bufs=2, name=f"rln_{j}")
                nc.scalar.activation(rln[:], rcat_sb[:], LN, scale=1.0)
                rr = work.tile([HPC, TJ], BF16, tag="rrcat", bufs=2, name=f"rrc_{j}")
                nc.scalar.activation(rr[:], rln[:], EXP, scale=-1.0)
                tail_state[j] = rr

            pend(1, recip)

            for p in range(NP):
                def mk(p):
                    def rbosc():
                        rr = tail_state[j]
                        rb_ps = psA.tile([128, TJ], F32, tag="mm512", bufs=2,
                                         name=f"rb_{p}_{j}")
                        nc.tensor.matmul(
                            rb_ps[:], selbc_sb[:, p * 128 : (p + 1) * 128], rr[:],
                            start=True, stop=True,
                        )
                        nc.vector.tensor_mul(osc_sb[p][:], osb_sb[p][:], rb_ps[:])
                    return rbosc
                pend(1, mk(p), key=("osc", p, j))

        def push_y(j, osc_sb):
            for m in range(EC):
                def mk(m):
                    def ychunk():
                        y_ps = psA.tile([128, TJ], F32, tag="mm512", bufs=2,
                                        name=f"y_{m}_{j}")
                        for p in range(NP):
                            nc.tensor.matmul(
                                y_ps[:], wp_sb[p][:, m * 128 : (m + 1) * 128],
                                osc_sb[p][:],
                                start=(p == 0), stop=(p == NP - 1),
                            )
                        y_sb = work.tile([128, TJ], F32, tag="y", bufs=2,
                                         name=f"ysb_{m}_{j}")
                        nc.vector.tensor_copy(y_sb[:], y_ps[:])
                        r0 = (j * EC + m) * 128
                        nc.sync.dma_start(out=yTt[r0 : r0 + 128, :], in_=y_sb[:])
                    return ychunk
                pend(2, mk(m))

        tail_state = {}

        # ---------------------------------------------------------------
        # Attention cell for (p, j): ACT-paced i-loop with filler pumping
        # ---------------------------------------------------------------
        def emit_attention(p, j):
            jt = slice(j * TJ, (j + 1) * TJ)
            ivals = []
            for i in range(NSI):
                types = [btab[i][4 * j + bl] for bl in range(4)]
                if all(t == "skip" for t in types):
                    continue
                ivals.append((i, types))
            n_i = len(ivals)

            require(("xq", p, j))

            o_ps = [
                psB.tile([65, TJ], F32, tag=f"ops{hh}", name=f"ops{hh}_{p}_{j}", bufs=1)
                for hh in range(2)
            ]
            touched = [[False] * 4, [False] * 4]
            sts = [None] * n_i     # (st_tile, u_tile, c0)

            def emit_pair(k):
                i, types = ivals[k]
                c0 = next(bl for bl in range(4) if types[bl] != "skip")
                require(("xk", p, i // 4))
                st = psA.tile([128, 2 * TJ], F32, tag="st", bufs=2)
                for hh in range(2):
                    hr = slice(hh * 64, (hh + 1) * 64)
                    nc.tensor.matmul(
                        st[:, hh * TJ + c0 * 128 : (hh + 1) * TJ],
                        xk_t[p][i // 4][hr, (i % 4) * 128 : (i % 4 + 1) * 128],
                        xq_t[p][j][hr, c0 * 128 : TJ],
                        start=True, stop=True,
                    )
                sts[k] = (st, None, c0)

            def emit_exp(k):
                st, _, c0 = sts[k]
                u = work.tile([128, 2 * TJ], BF16, tag="u", bufs=4)
                nc.scalar.activation(
                    u[:].rearrange("p (g c) -> p g c", g=2)[:, :, c0 * 128 : TJ],
                    st[:].rearrange("p (g c) -> p g c", g=2)[:, :, c0 * 128 : TJ],
                    EXP, scale=1.0 / (32.0 * 256.0),  # wq,wk pre-scaled x16 each
                )
                sts[k] = (st, u, c0)

            def emit_av(k):
                i, types = ivals[k]
                _, u, c0 = sts[k]
                require(("xv", i))
                for hh in range(2):
                    h = 2 * p + hh
                    uo = hh * TJ
                    runs = []  # (bl0, bl1, src_ap)
                    bl = c0
                    while bl < 4:
                        if types[bl] == "dense":
                            b2 = bl
                            while b2 + 1 < 4 and types[b2 + 1] == "dense":
                                b2 += 1
                            runs.append((bl, b2 + 1,
                                         u[:, uo + bl * 128 : uo + (b2 + 1) * 128]))
                            bl = b2 + 1
                        elif types[bl] == "skip":
                            bl += 1
                        else:
                            mt = work.tile([128, 128], BF16, tag="mfix", bufs=4)
                            nc.vector.tensor_mul(
                                mt[:], u[:, uo + bl * 128 : uo + (bl + 1) * 128],
                                pat_sb[types[bl]][:],
                            )
                            runs.append((bl, bl + 1, mt[:]))
                            bl += 1
                    lhs_v = xv_sb[i][:, h * 65 : h * 65 + 65]
                    for ri, (b0, b1, src) in enumerate(runs):
                        first = all(not touched[hh][b] for b in range(b0, b1))
                        assert first == any(
                            not touched[hh][b] for b in range(b0, b1)
                        ), "mask blocks: mixed touch state inside a run"
                        last = (k == n_i - 1) and (ri == len(runs) - 1)
                        nc.tensor.matmul(
                            o_ps[hh][:, b0 * 128 : b1 * 128],
                            lhs_v, src,
                            start=first, stop=last,
                            skip_group_check=True,
                        )
                        for b in range(b0, b1):
                            touched[hh][b] = True
                sts[k] = None

            # pipelined i-loop, two steps per "mode era" to halve the
            # 64<->128 row-tiling mode switches: [pair pair] [exp exp]
            # [fillers AV AV]
            emit_pair(0)
            if n_i > 1:
                emit_pair(1)
            emit_exp(0)
            if n_i > 1:
                emit_exp(1)
            k = 0
            while k < n_i:
                k2 = min(k + 2, n_i)
                for kk in range(k + 2, min(k + 4, n_i)):
                    emit_pair(kk)
                for kk in range(k + 2, min(k + 4, n_i)):
                    emit_exp(kk)
                # AV(k) right after the pairs: the 64->128 mode switch lands
                # on its cheap 65-col LDWEIGHTS, and exp(k) finished last era
                emit_av(k)
                pump(k2 - k)
                for kk in range(k + 1, k2):
                    emit_av(kk)
                k = k2

            require(("osc", p, j - 2))  # osb buffer of j-2 must be fully consumed
            # stage row sums (bf16, 1 lane) + o rows (bf16) so o_ps frees
            for hh in range(2):
                h = 2 * p + hh
                rsb = work.tile([1, TJ], BF16, tag="rsb", bufs=4)
                nc.vector.tensor_copy(rsb[:], o_ps[hh][64:65, :])
                nc.sync.dma_start(out=rcat_cur[0][h : h + 1, :], in_=rsb[:])
                nc.vector.tensor_copy(
                    osb_cur[0][p][hh * 64 : (hh + 1) * 64, :], o_ps[hh][0:64, :]
                )

        rcat_cur = [None]
        osb_cur = [None]

        # ---------------------------------------------------------------
        # Prologue
        # ---------------------------------------------------------------
        # warm the ACT exp table while DMAs run
        nc.vector.memset(dummy_sb[:], 0.0)
        nc.scalar.activation(dummy_sb[:, 0:1], dummy_sb[:, 1:2], EXP, scale=1.0)
        for i in range(NSI):
            nc.vector.memset(
                xv_sb[i][:].rearrange("p (h x) -> p h x", x=65)[:, :, 64:65], 1.0
            )

        alloc_stream_tiles(0)
        # first-needed DMAs first: wq+qs(0), then wk+ks(0), wv+vs(0)
        for c in range(EC // 2):
            for sub in range(2):
                e = 2 * c + sub
                nc.sync.dma_start(out=wq_sb[c][:, sub * DC : (sub + 1) * DC],
                                  in_=wq[e * 128 : (e + 1) * 128, :])
            dma_pair(qs_j[0][c], qTt, 0, c, TJ)
        for c in range(EC // 2):
            for sub in range(2):
                e = 2 * c + sub
                nc.sync.dma_start(out=wk_sb[c][:, sub * DC : (sub + 1) * DC],
                                  in_=wk[e * 128 : (e + 1) * 128, :])
            dma_pair(ks_j[0][c], kTt, 0, c, TJ)
        for e in range(EC):
            nc.sync.dma_start(out=wv_sb[e][:], in_=wv[e * 128 : (e + 1) * 128, :])
            nc.sync.dma_start(out=vs_j[0][e][:],
                              in_=vTt[e * 128 : (e + 1) * 128, :])
        nc.sync.dma_start(out=selbc_sb[:], in_=selbc[:])
        for u in range(n_pat):
            nc.sync.dma_start(out=pat_sb[u][:], in_=pat[u * 128 : (u + 1) * 128, :])
        for p in range(NP):
            nc.sync.dma_start(out=wp_sb[p][:], in_=wpT[p * 128 : (p + 1) * 128, :])
        alloc_stream_tiles(1)
        emit_stream_dmas(1)

        # inline p0 projections for j=0, rest queued
        push_qk_chain(0, 0)
        require(("xk", 0, 0))
        push_proj(0, first_p_inline=True)

        # ---------------------------------------------------------------
        # Main loop
        # ---------------------------------------------------------------
        for j in range(NJ):
            if j + 1 < NJ:
                push_proj(j + 1)

            rcat_cur[0] = work.tile([HPC, TJ], BF16, tag="rcat", bufs=2, name=f"rcat_{j}")
            osb_cur[0] = [
                work.tile([128, TJ], BF16, tag=f"osb{p}", bufs=2, name=f"osb{p}_{j}")
                for p in range(NP)
            ]
            osc_sb = osc_sb_all[j % 2]
            rcat_sb = rcat_cur[0]
            osb_sb = osb_cur[0]

            for p in range(NP):
                if p == 1:
                    release_pending()
                emit_attention(p, j)

            # all proj(j) closures are drained by now (att(j) p=3 required
            # ("xk", 3, j)), so the buffer-recycling DMA is safe to emit
            if j + 2 < NJ:
                alloc_stream_tiles(j + 2)
                emit_stream_dmas(j + 2)

            push_tail(j, rcat_sb, osb_sb, osc_sb)
            push_y(j, osc_sb)

        # drain everything left (tail + y of the last tiles)
        release_pending()
        while fillers:
            _run_one()

    _split_multi_waits(nc)
    return nc


_SELBC = np.zeros((HPC, DC), NPBF16)
for _p in range(HPC // 2):
    _SELBC[2 * _p, _p * 128 : _p * 128 + 64] = 1.0
    _SELBC[2 * _p + 1, _p * 128 + 64 : _p * 128 + 128] = 1.0

_CACHE = {}


def _get_program(mask):
    key = np.asarray(mask, dtype=bool).tobytes()
    prog = _CACHE.get(key)
    if prog is None:
        _install_patches()
        btab, patterns = _classify_mask(mask)
        nc = _build(btab, len(patterns))
        prog = (nc, patterns)
        _CACHE[key] = prog
    return prog


def _prepare(k, q, v, mask, Wk, Wq, Wv, Wp):
    """Build (cached) the SPMD program and the 8 per-core input maps."""
    k = np.asarray(k, np.float32)
    q = np.asarray(q, np.float32)
    v = np.asarray(v, np.float32)
    Wk = np.asarray(Wk, np.float32)
    Wq = np.asarray(Wq, np.float32)
    Wv = np.asarray(Wv, np.float32)
    Wp = np.asarray(Wp, np.float32)

    nc, patterns = _get_program(mask)
    patflat = np.ascontiguousarray(patterns.reshape(-1, 128))

    def tr_tiled(x, dt):  # [T, E] f32 -> [NJ*EC*128, TJ], tile (j,e) contiguous
        xt = np.ascontiguousarray(x.astype(dt).T)          # [E, T]
        xt = xt.reshape(EC, 128, NJ, TJ).transpose(2, 0, 1, 3)
        return np.ascontiguousarray(xt.reshape(NJ * EC * 128, TJ))

    def wcat(W, half, dt, scale=1.0):  # [H, E, D] -> [E, 512] for 8 heads
        w = W[half * HPC : (half + 1) * HPC].transpose(1, 0, 2).reshape(E, DC)
        return np.ascontiguousarray(w * scale).astype(dt)

    in_maps = []
    for c in range(8):
        b, half = divmod(c, 2)
        off = half * DC
        in_maps.append(
            {
                "qTt": tr_tiled(q[b], NPFP8),
                "kTt": tr_tiled(k[b], NPFP8),
                "vTt": tr_tiled(v[b], NPBF16),
                # wq/wk pre-scaled x16 for fp8 range; exp scale divides it out
                "wq": wcat(Wq, half, NPFP8, 16.0),
                "wk": wcat(Wk, half, NPFP8, 16.0),
                "wv": wcat(Wv, half, NPBF16),
                "wpT": np.ascontiguousarray(Wp[:, off : off + DC].T).astype(NPBF16),
                "pat": patflat,
                "selbc": _SELBC,
            }
        )
    return nc, in_maps


def kernel(k, q, v, mask, Wk, Wq, Wv, Wp, bp):
    bp = np.asarray(bp, np.float32)
    nc, in_maps = _prepare(k, q, v, mask, Wk, Wq, Wv, Wp)
    res = run_bass_kernel_spmd(nc, in_maps, list(range(8)))
    out = np.empty((B, T, E), np.float32)
    for b in range(B):
        yt = res.results[2 * b]["yTt"] + res.results[2 * b + 1]["yTt"]
        # [NJ*EC*128, TJ] -> [E, T]
        yt = yt.reshape(NJ, EC, 128, TJ).transpose(1, 2, 0, 3).reshape(E, T)
        out[b] = yt.T + bp[None, :]
    return out


# revision 26
# speedup vs baseline: 1.2147x; 1.0170x over previous
"""Multi-head causal attention (B=4, T=2048, E=1024, H=16, D=64) on 8 trn2
NeuronCores via Bass/Tile.

Sharding: core c handles batch b = c//2 and heads [half*8, half*8+8), half =
c%2. Each core computes its 8 heads' attention and a partial output
projection; the host sums the two half partials per batch, transposes, and
adds the bias.

On-device layout is "transposed": activations are [feature, token] so every
matmul contracts over the partition dim. Softmax denominators come from a
ones-column appended to the stationary V operand (M=65 matmuls); masking is
applied block-wise (128x128) with patterns derived from the actual mask input
at build time. No max-subtraction is needed: scores are ~N(0, 0.083^2).

This version software-pipelines the whole kernel: the attention i-loop is
ACT(exp)-paced, so projection matmuls for the next t-tile, output-projection
matmuls for the previous t-tile, and softmax-tail work are injected as
"filler" closures between attention steps to keep the PE busy. Inputs are
pre-tiled host-side so every DMA moves one contiguous 128KB block.
"""
import numpy as np
import ml_dtypes
from collections import deque
from contextlib import ExitStack

import concourse.bass as bass
import concourse.mybir as mybir
import concourse.tile as tile
from concourse.bass_utils import run_bass_kernel_spmd
from concourse.vector_clock import ScopedClock

BF16 = mybir.dt.bfloat16
F32 = mybir.dt.float32
NPBF16 = ml_dtypes.bfloat16
NPFP8 = ml_dtypes.float8_e4m3fn

B, T, E, H, D = 4, 2048, 1024, 16, 64
HPC = 8            # heads per core
DC = HPC * D       # 512: stacked head dim per core
TJ = 512           # t tile (matmul free dim)
NJ = T // TJ       # 4
SI = 128           # s tile (psum partition dim)
NSI = T // SI      # 16
EC = E // 128      # 8 e-chunks
NP = HPC // 2      # 4 head pairs

# ---------------------------------------------------------------------------
# Workarounds for this walrus build: at most ONE sync wait per instruction.
# ---------------------------------------------------------------------------
_PATCHED = False


def _patched_drain_and_barrier(self, tick_clock, wait_clock):
    drain_inst = self.nc.sync.drain(fusable=False)
    wait_clock.add_sem_waits(
        drain_inst.ins, ScopedClock({None: tick_clock.global_clock})
    )
    si = drain_inst.ins.sync_info
    if si is not None and len(si.on_wait) > 1:
        waits = list(si.on_wait)
        drain_inst.ins.sync_info = mybir.SyncInfo(
            on_wait=waits[:1], on_update=list(si.on_update)
        )
        for ofs in range(1, len(waits)):
            extra = self.nc.sync.drain(fusable=False)
            extra.ins.sync_info = mybir.SyncInfo(
                on_wait=waits[ofs : ofs + 1], on_update=[]
            )
    self.nc.all_engine_barrier()
    assert self.sems is not None
    popped = self.nc._tile_sem_poison_stack.pop()
    assert popped is self._sem_poison
    self.nc.clear_and_free_semaphores(list(self.sems.allocated().values()))
    self.nc.all_engine_barrier()


def _install_patches():
    global _PATCHED
    if _PATCHED:
        return
    tile.TileContext._drain_and_barrier = _patched_drain_and_barrier
    _PATCHED = True


def _make_carrier(nc, engine, wait):
    """Wait-only EventSemaphore on `engine` (cheap: ~70ns, no pipe flush)."""
    ev = mybir.InstEventSemaphore(name=f"W-{nc.next_id()}", ins=[], outs=[])
    ev.engine = engine
    ev.sync_info = mybir.SyncInfo(on_wait=[wait], on_update=[])
    return ev


_ENGINE_SEM = {
    "EngineType.PE": "PE",
    "EngineType.DVE": "DVE",
    "EngineType.Activation": "Activation",
    "EngineType.SP": "SP",
    "EngineType.Pool": "Pool",
}
# engines with in-order issue AND in-order completion for these inst types:
# a wait on the engine's own completion sem is redundant. Ldweights excluded
# (the PE reorder window pulls it ahead of in-flight matmuls).
_DROPPABLE = (
    "InstMatmult", "InstActivation", "InstTensorTensor", "InstTensorCopy",
    "InstTensorReduce", "InstMemset", "InstReciprocal", "InstDMACopy",
    "InstCopyPredicated", "InstTensorScalarPtr", "InstTensorScalar",
    "InstCast", "InstDveOp", "InstCustomDve",
)


def _split_multi_waits(nc):
    for bbw in list(nc.bb_map.values()):
        bb = bbw.bb
        insts = bb.instructions
        if not any(
            getattr(i, "sync_info", None) is not None and len(i.sync_info.on_wait) > 1
            for i in insts
        ):
            continue
        out = []
        for inst in insts:
            si = getattr(inst, "sync_info", None)
            waits = list(si.on_wait) if si is not None else []
            if len(waits) > 1:
                own = _ENGINE_SEM.get(str(inst.engine))
                tn = type(inst).__name__
                if own is not None and tn.startswith(_DROPPABLE):
                    waits = [
                        w for w in waits
                        if w.ant_name.rsplit("_", 1)[0] != own
                    ] or waits[-1:]
            if len(waits) > 1:
                for w in waits[:-1]:
                    out.append(_make_carrier(nc, inst.engine, w))
                waits = waits[-1:]
            if si is not None and list(si.on_wait) != waits:
                inst.sync_info = mybir.SyncInfo(
                    on_wait=waits, on_update=list(si.on_update)
                )
            out.append(inst)
        insts[:] = out


# ---------------------------------------------------------------------------
# Mask analysis (host side, 128x128 blocks).
# ---------------------------------------------------------------------------
def _classify_mask(mask):
    """mask: [T, T] bool, mask[t, s]=True means masked (score -> -inf).

    Returns (btab, patterns): btab[i][jj] in {'skip', 'dense', int u};
    patterns[u] is a [128,128] bf16 multiplier in [s, t] orientation."""
    nb = T // 128
    m = np.asarray(mask, dtype=bool)
    patterns = []
    index = {}
    btab = [[None] * nb for _ in range(nb)]
    for i in range(nb):          # s block
        for jj in range(nb):     # t block
            sub = m[jj * 128 : (jj + 1) * 128, i * 128 : (i + 1) * 128]  # [t, s]
            if sub.all():
                btab[i][jj] = "skip"
            elif not sub.any():
                btab[i][jj] = "dense"
            else:
                pat = (~sub).T.astype(NPBF16)  # [s, t] multiplier
                key = pat.tobytes()
                if key not in index:
                    index[key] = len(patterns)
                    patterns.append(pat)
                btab[i][jj] = index[key]
    if not patterns:
        patterns.append(np.ones((128, 128), NPBF16))
    return btab, np.stack(patterns)


# ---------------------------------------------------------------------------
# Kernel builder (SPMD program, identical on all 8 cores).
# ---------------------------------------------------------------------------
def _build(btab, n_pat):
    nc = bass.Bass()
    FP8 = mybir.dt.float8e4
    # pre-tiled streams: tile (j, e) is rows (j*EC+e)*128 .. +128, contiguous
    # q/k are fp8 (DoubleRow matmuls; errors attenuate through softmax)
    qTt = nc.declare_dram_parameter("qTt", [128, NJ * EC * TJ], FP8, isOutput=False)
    kTt = nc.declare_dram_parameter("kTt", [128, NJ * EC * TJ], FP8, isOutput=False)
    vTt = nc.declare_dram_parameter("vTt", [128, NJ * EC * TJ], BF16, isOutput=False)
    wq = nc.declare_dram_parameter("wq", [128, EC * DC], FP8, isOutput=False)
    wk = nc.declare_dram_parameter("wk", [128, EC * DC], FP8, isOutput=False)
    wv = nc.declare_dram_parameter("wv", [128, EC * DC], BF16, isOutput=False)
    wpT = nc.declare_dram_parameter("wpT", [DC, E], BF16, isOutput=False)
    pat = nc.declare_dram_parameter("pat", [n_pat * 128, 128], BF16, isOutput=False)
    selbc = nc.declare_dram_parameter("selbc", [HPC, DC], BF16, isOutput=False)
    # pre-tiled output: tile (j, m) is rows (j*EC+m)*128 .. +128
    yTt = nc.declare_dram_parameter("yTt", [NJ * EC * 128, TJ], F32, isOutput=True)

    EXP = mybir.ActivationFunctionType.Exp
    LN = mybir.ActivationFunctionType.Ln

    with ExitStack() as ctx:
        tc = ctx.enter_context(tile.TileContext(nc))
        # SBUF pools
        consts = ctx.enter_context(tc.tile_pool(name="consts", bufs=1))
        streams = ctx.enter_context(tc.tile_pool(name="streams", bufs=1))
        acts = ctx.enter_context(tc.tile_pool(name="acts", bufs=1))
        work = ctx.enter_context(tc.tile_pool(name="work", bufs=1))
        # PSUM pools
        psA = ctx.enter_context(tc.tile_pool(name="psA", bufs=1, space="PSUM"))
        psB = ctx.enter_context(tc.tile_pool(name="psB", bufs=1, space="PSUM"))

        # ---- constants ----
        # weights as single partition-major tiles; e-chunk c lives at
        # cols [c*DC, (c+1)*DC)
        wq_sb = consts.tile([128, EC * DC], FP8, tag="wq", name="wq", bufs=1)
        wk_sb = consts.tile([128, EC * DC], FP8, tag="wk", name="wk", bufs=1)
        wv_sb = consts.tile([128, EC * DC], BF16, tag="wv", name="wv", bufs=1)
        wp_sb = [consts.tile([128, E], BF16, tag=f"wp{p}", name=f"wp{p}", bufs=1) for p in range(NP)]
        pat_sb = [consts.tile([128, 128], BF16, tag=f"pat{u}", name=f"pat{u}", bufs=1) for u in range(n_pat)]
        selbc_sb = consts.tile([HPC, DC], BF16, tag="selbc", name="selbc", bufs=1)
        dummy_sb = consts.tile([1, 2], BF16, tag="dummy", name="dummy", bufs=1)

        # ---- persistent activations (per (p, j) tiles so pipelined writes
        # to tile j+1 never alias reads of tile j) ----
        xq_t = [[acts.tile([128, TJ], BF16, tag=f"xq{p}_{j}", name=f"xq{p}_{j}", bufs=1)
                 for j in range(NJ)] for p in range(NP)]
        xk_t = [[acts.tile([128, TJ], BF16, tag=f"xk{p}_{j}", name=f"xk{p}_{j}", bufs=1)
                 for j in range(NJ)] for p in range(NP)]
        # xv tiles: per s-tile, heads laid out as 8 x (64 cols xv | 1 col ones)
        xv_sb = [acts.tile([128, HPC * 65], BF16, tag=f"xv{i}", name=f"xv{i}", bufs=1) for i in range(NSI)]
        osc_sb_all = [
            [acts.tile([128, TJ], BF16, tag=f"osc{p}_{jj}", name=f"osc{p}_{jj}", bufs=1)
             for p in range(NP)]
            for jj in range(2)
        ]

        # ---------------------------------------------------------------
        # Filler queue: closures emitting ~1-2 PE matmuls (+ their DVE/DMA
        # tails). `require(key)` force-drains through a named closure.
        # ---------------------------------------------------------------
        fillers = deque()        # entries: (cost, fn)
        ready_idx = {}           # key -> push counter of last closure for key
        drained = [0]            # count of executed closures
        pushed = [0]
        debt = [0.0]

        def push(cost, fn, key=None):
            fillers.append((cost, fn))
            pushed[0] += 1
            if key is not None:
                ready_idx[key] = pushed[0]

        def _run_one():
            cost, fn = fillers.popleft()
            fn()
            drained[0] += 1
            return cost

        def pump(units):
            debt[0] += units
            while fillers and debt[0] >= fillers[0][0]:
                debt[0] -= _run_one()

        def require(key):
            idx = ready_idx.get(key, 0)
            while drained[0] < idx:
                _run_one()

        # ---------------------------------------------------------------
        # DMA emission
        # ---------------------------------------------------------------
        def dma_split(dst, src_ap):
            # split by partition quarter so 4 DMA queues work in parallel
            for r in range(0, 128, 32):
                nc.sync.dma_start(out=dst[r : r + 32, :], in_=src_ap[r : r + 32, :])

        def emit_stream_dmas(j):
            w = EC * TJ
            dma_split(qs_j[j][:], qTt[:, j * w : (j + 1) * w])
            dma_split(ks_j[j][:], kTt[:, j * w : (j + 1) * w])
            dma_split(vs_j[j][:], vTt[:, j * w : (j + 1) * w])

        # explicit per-j stream tile handles (2 buffer sets, alternating);
        # q/k as fp8 chunk-pair tiles, v as bf16 per-chunk tiles
        qs_j, ks_j, vs_j = {}, {}, {}

        def alloc_stream_tiles(j):
            qs_j[j] = streams.tile([128, EC * TJ], FP8, tag="qs", name=f"qs_{j}", bufs=2)
            ks_j[j] = streams.tile([128, EC * TJ], FP8, tag="ks", name=f"ks_{j}", bufs=2)
            vs_j[j] = streams.tile([128, EC * TJ], BF16, tag="vs", name=f"vs_{j}", bufs=2)

        # ---------------------------------------------------------------
        # Projection chain closures
        # ---------------------------------------------------------------
        DR = mybir.MatmulPerfMode.DoubleRow

        def _dr_mm(ps, w_sb, s_tile, p, c, start, stop):
            pc = slice(p * 128, (p + 1) * 128)
            lhsT = w_sb[:, 2 * c * DC : (2 * c + 2) * DC].rearrange(
                "r (k m) -> r k m", k=2)[:, :, pc]
            rhs = s_tile[:, 2 * c * TJ : (2 * c + 2) * TJ].rearrange(
                "r (k n) -> r k n", k=2)
            nc.tensor.matmul(ps[:], lhsT, rhs, start=start, stop=stop,
                             perf_mode=DR)

        def push_qk_chain(p, j):
            """xq and xk chains for (p, j): DoubleRow fp8, 2 closures each."""
            st_q = {}

            def q1():
                ps = psA.tile([128, TJ], F32, tag="mm512", bufs=2, name=f"xqp{p}_{j}")
                st_q['ps'] = ps
                for c in range(2):
                    _dr_mm(ps, wq_sb, qs_j[j], p, c, c == 0, False)

            def q2():
                ps = st_q['ps']
                for c in range(2, EC // 2):
                    _dr_mm(ps, wq_sb, qs_j[j], p, c, False, c == EC // 2 - 1)
                nc.vector.tensor_copy(xq_t[p][j][:], ps[:])

            st_k = {}

            def k1():
                ps = psA.tile([128, TJ], F32, tag="mm512", bufs=2, name=f"xkp{p}_{j}")
                st_k['ps'] = ps
                for c in range(2):
                    _dr_mm(ps, wk_sb, ks_j[j], p, c, c == 0, False)

            def k2():
                ps = st_k['ps']
                for c in range(2, EC // 2):
                    _dr_mm(ps, wk_sb, ks_j[j], p, c, False, c == EC // 2 - 1)
                nc.vector.tensor_copy(xk_t[p][j][:], ps[:])

            push(1, q1)
            push(1, q2, key=("xq", p, j))
            push(1, k1)
            push(1, k2, key=("xk", p, j))

        def push_xv_chain(loc, j):
            si = 4 * j + loc
            st = {}

            def _vmm(ps, e, start, stop):
                lhsT = vs_j[j][:, e * TJ + loc * 128 : e * TJ + (loc + 1) * 128]
                rhs = wv_sb[:, e * DC : (e + 1) * DC]
                nc.tensor.matmul(ps[:], lhsT, rhs, start=start, stop=stop)

            def v1():
                ps = psA.tile([128, DC], F32, tag="mm512", bufs=2, name=f"xvp{si}")
                st['ps'] = ps
                for e in range(4):
                    _vmm(ps, e, e == 0, False)

            def v2():
                ps = st['ps']
                for e in range(4, EC):
                    _vmm(ps, e, False, e == EC - 1)
                nc.vector.tensor_copy(
                    xv_sb[si][:].rearrange("p (h x) -> p h x", x=65)[:, :, 0:64],
                    ps[:].rearrange("p (h d) -> p h d", h=HPC),
                )

            push(2, v1)
            push(2, v2, key=("xv", si))

        def push_proj(j, first_p_inline=False):
            """All projection chains for t-tile j, p0's q/k first."""
            order = []
            if not first_p_inline:
                order.append(("qk", 0))
            order += [("xv", loc) for loc in range(4)]
            order += [("qk", p) for p in range(1, NP)]
            for kind, a in order:
                if kind == "qk":
                    push_qk_chain(a, j)
                else:
                    push_xv_chain(a, j)

        # ---------------------------------------------------------------
        # Softmax tail + output projection closures
        # ---------------------------------------------------------------
        pending_tail = []    # (cost, fn, key) released at p==1 of next att(j)

        def pend(cost, fn, key=None):
            pending_tail.append((cost, fn, key))

        def release_pending():
            for cost, fn, key in pending_tail:
                push(cost, fn, key=key)
            pending_tail.clear()

        def push_tail(j, rcat_sb, osb_sb, osc_sb):
            def recip():
                # 1/r as exp(-ln r) on ACT: exp+ln share one table set; the
                # DVE reciprocal is lane-starved on [8, TJ] (3.3us).
                rln = work.tile([HPC, TJ], F32, tag="rrcat32", bufs=2, name=f"rln_{j}")
                nc.scalar.activation(rln[:], rcat_sb[:], LN, scale=1.0)
                rr = work.tile([HPC, TJ], BF16, tag="rrcat", bufs=2, name=f"rrc_{j}")
                nc.scalar.activation(rr[:], rln[:], EXP, scale=-1.0)
                tail_state[j] = rr

            pend(1, recip)

            for p in range(NP):
                def mk(p):
                    def rbosc():
                        rr = tail_state[j]
                        rb_ps = psA.tile([128, TJ], F32, tag="mm512", bufs=2,
                                         name=f"rb_{p}_{j}")
                        nc.tensor.matmul(
                            rb_ps[:], selbc_sb[:, p * 128 : (p + 1) * 128], rr[:],
                            start=True, stop=True,
                        )
                        nc.vector.tensor_mul(osc_sb[p][:], osb_sb[p][:], rb_ps[:])
                    return rbosc
                pend(1, mk(p), key=("osc", p, j))

        def push_y(j, osc_sb):
            for m in range(EC):
                def mk(m):
                    def ychunk():
                        y_ps = psA.tile([128, TJ], F32, tag="mm512", bufs=2,
                                        name=f"y_{m}_{j}")
                        for p in range(NP):
                            nc.tensor.matmul(
                                y_ps[:], wp_sb[p][:, m * 128 : (m + 1) * 128],
                                osc_sb[p][:],
                                start=(p == 0), stop=(p == NP - 1),
                            )
                        y_sb = work.tile([128, TJ], F32, tag="y", bufs=2,
                                         name=f"ysb_{m}_{j}")
                        nc.vector.tensor_copy(y_sb[:], y_ps[:])
                        r0 = (j * EC + m) * 128
                        nc.sync.dma_start(out=yTt[r0 : r0 + 128, :], in_=y_sb[:])
                    return ychunk
                pend(2, mk(m))

        tail_state = {}

        # ---------------------------------------------------------------
        # Attention cell for (p, j): ACT-paced i-loop with filler pumping
        # ---------------------------------------------------------------
        def emit_attention(p, j):
            jt = slice(j * TJ, (j + 1) * TJ)
            ivals = []
            for i in range(NSI):
                types = [btab[i][4 * j + bl] for bl in range(4)]
                if all(t == "skip" for t in types):
                    continue
                ivals.append((i, types))
            n_i = len(ivals)

            require(("xq", p, j))

            o_ps = [
                psB.tile([65, TJ], F32, tag=f"ops{hh}", name=f"ops{hh}_{p}_{j}", bufs=1)
                for hh in range(2)
            ]
            touched = [[False] * 4, [False] * 4]
            sts = [None] * n_i     # (st_tile, u_tile, c0)

            def emit_pair(k):
                i, types = ivals[k]
                c0 = next(bl for bl in range(4) if types[bl] != "skip")
                require(("xk", p, i // 4))
                st = psA.tile([128, 2 * TJ], F32, tag="st", bufs=2)
                for hh in range(2):
                    hr = slice(hh * 64, (hh + 1) * 64)
                    nc.tensor.matmul(
                        st[:, hh * TJ + c0 * 128 : (hh + 1) * TJ],
                        xk_t[p][i // 4][hr, (i % 4) * 128 : (i % 4 + 1) * 128],
                        xq_t[p][j][hr, c0 * 128 : TJ],
                        start=True, stop=True,
                    )
                sts[k] = (st, None, c0)

            def emit_exp(k):
                st, _, c0 = sts[k]
                u = work.tile([128, 2 * TJ], BF16, tag="u", bufs=4)
                nc.scalar.activation(
                    u[:].rearrange("p (g c) -> p g c", g=2)[:, :, c0 * 128 : TJ],
                    st[:].rearrange("p (g c) -> p g c", g=2)[:, :, c0 * 128 : TJ],
                    EXP, scale=1.0 / (32.0 * 256.0),  # wq,wk pre-scaled x16 each
                )
                sts[k] = (st, u, c0)

            def emit_av(k):
                i, types = ivals[k]
                _, u, c0 = sts[k]
                require(("xv", i))
                for hh in range(2):
                    h = 2 * p + hh
                    uo = hh * TJ
                    runs = []  # (bl0, bl1, src_ap)
                    bl = c0
                    while bl < 4:
                        if types[bl] == "dense":
                            b2 = bl
                            while b2 + 1 < 4 and types[b2 + 1] == "dense":
                                b2 += 1
                            runs.append((bl, b2 + 1,
                                         u[:, uo + bl * 128 : uo + (b2 + 1) * 128]))
                            bl = b2 + 1
                        elif types[bl] == "skip":
                            bl += 1
                        else:
                            mt = work.tile([128, 128], BF16, tag="mfix", bufs=4)
                            nc.vector.tensor_mul(
                                mt[:], u[:, uo + bl * 128 : uo + (bl + 1) * 128],
                                pat_sb[types[bl]][:],
                            )
                            runs.append((bl, bl + 1, mt[:]))
                            bl += 1
                    lhs_v = xv_sb[i][:, h * 65 : h * 65 + 65]
                    for ri, (b0, b1, src) in enumerate(runs):
                        first = all(not touched[hh][b] for b in range(b0, b1))
                        assert first == any(
                            not touched[hh][b] for b in range(b0, b1)
                        ), "mask blocks: mixed touch state inside a run"
                        last = (k == n_i - 1) and (ri == len(runs) - 1)
                        nc.tensor.matmul(
                            o_ps[hh][:, b0 * 128 : b1 * 128],
                            lhs_v, src,
                            start=first, stop=last,
                            skip_group_check=True,
                        )
                        for b in range(b0, b1):
                            touched[hh][b] = True
                sts[k] = None

            # pipelined i-loop, two steps per "mode era" to halve the
            # 64<->128 row-tiling mode switches: [pair pair] [exp exp]
            # [fillers AV AV]
            emit_pair(0)
            if n_i > 1:
                emit_pair(1)
            emit_exp(0)
            if n_i > 1:
                emit_exp(1)
            k = 0
            while k < n_i:
                k2 = min(k + 2, n_i)
                for kk in range(k + 2, min(k + 4, n_i)):
                    emit_pair(kk)
                for kk in range(k + 2, min(k + 4, n_i)):
                    emit_exp(kk)
                # AV(k) right after the pairs: the 64->128 mode switch lands
                # on its cheap 65-col LDWEIGHTS, and exp(k) finished last era
                emit_av(k)
                pump(k2 - k)
                for kk in range(k + 1, k2):
                    emit_av(kk)
                k = k2

            require(("osc", p, j - 2))  # osb buffer of j-2 must be fully consumed
            # stage row sums (bf16, 1 lane) + o rows (bf16) so o_ps frees
            for hh in range(2):
                h = 2 * p + hh
                rsb = work.tile([1, TJ], BF16, tag="rsb", bufs=4)
                nc.vector.tensor_copy(rsb[:], o_ps[hh][64:65, :])
                nc.sync.dma_start(out=rcat_cur[0][h : h + 1, :], in_=rsb[:])
                nc.vector.tensor_copy(
                    osb_cur[0][p][hh * 64 : (hh + 1) * 64, :], o_ps[hh][0:64, :]
                )

        rcat_cur = [None]
        osb_cur = [None]

        # ---------------------------------------------------------------
        # Prologue
        # ---------------------------------------------------------------
        # warm the ACT exp table while DMAs run
        nc.vector.memset(dummy_sb[:], 0.0)
        nc.scalar.activation(dummy_sb[:, 0:1], dummy_sb[:, 1:2], EXP, scale=1.0)
        for i in range(NSI):
            nc.vector.memset(
                xv_sb[i][:].rearrange("p (h x) -> p h x", x=65)[:, :, 64:65], 1.0
            )

        alloc_stream_tiles(0)
        # first-needed DMAs first: wq+qs(0), then wk+ks(0), wv+vs(0)
        w = EC * TJ
        dma_split(wq_sb[:], wq[:])
        dma_split(qs_j[0][:], qTt[:, 0:w])
        dma_split(wk_sb[:], wk[:])
        dma_split(ks_j[0][:], kTt[:, 0:w])
        dma_split(wv_sb[:], wv[:])
        dma_split(vs_j[0][:], vTt[:, 0:w])
        nc.sync.dma_start(out=selbc_sb[:], in_=selbc[:])
        for u in range(n_pat):
            nc.sync.dma_start(out=pat_sb[u][:], in_=pat[u * 128 : (u + 1) * 128, :])
        for p in range(NP):
            nc.sync.dma_start(out=wp_sb[p][:], in_=wpT[p * 128 : (p + 1) * 128, :])
        alloc_stream_tiles(1)
        emit_stream_dmas(1)

        # inline p0 projections for j=0, rest queued
        push_qk_chain(0, 0)
        require(("xk", 0, 0))
        push_proj(0, first_p_inline=True)

        # ---------------------------------------------------------------
        # Main loop
        # ---------------------------------------------------------------
        for j in range(NJ):
            if j + 1 < NJ:
                push_proj(j + 1)

            rcat_cur[0] = work.tile([HPC, TJ], BF16, tag="rcat", bufs=2, name=f"rcat_{j}")
            osb_cur[0] = [
                work.tile([128, TJ], BF16, tag=f"osb{p}", bufs=2, name=f"osb{p}_{j}")
                for p in range(NP)
            ]
            osc_sb = osc_sb_all[j % 2]
            rcat_sb = rcat_cur[0]
            osb_sb = osb_cur[0]

            for p in range(NP):
                if p == 1:
                    release_pending()
                emit_attention(p, j)

            # all proj(j) closures are drained by now (att(j) p=3 required
            # ("xk", 3, j)), so the buffer-recycling DMA is safe to emit
            if j + 2 < NJ:
                alloc_stream_tiles(j + 2)
                emit_stream_dmas(j + 2)

            push_tail(j, rcat_sb, osb_sb, osc_sb)
            push_y(j, osc_sb)

        # drain everything left (tail + y of the last tiles)
        release_pending()
        while fillers:
            _run_one()

    _split_multi_waits(nc)
    return nc


_SELBC = np.zeros((HPC, DC), NPBF16)
for _p in range(HPC // 2):
    _SELBC[2 * _p, _p * 128 : _p * 128 + 64] = 1.0
    _SELBC[2 * _p + 1, _p * 128 + 64 : _p * 128 + 128] = 1.0

_CACHE = {}


def _get_program(mask):
    key = np.asarray(mask, dtype=bool).tobytes()
    prog = _CACHE.get(key)
    if prog is None:
        _install_patches()
        btab, patterns = _classify_mask(mask)
        nc = _build(btab, len(patterns))
        prog = (nc, patterns)
        _CACHE[key] = prog
    return prog


def _prepare(k, q, v, mask, Wk, Wq, Wv, Wp):
    """Build (cached) the SPMD program and the 8 per-core input maps."""
    k = np.asarray(k, np.float32)
    q = np.asarray(q, np.float32)
    v = np.asarray(v, np.float32)
    Wk = np.asarray(Wk, np.float32)
    Wq = np.asarray(Wq, np.float32)
    Wv = np.asarray(Wv, np.float32)
    Wp = np.asarray(Wp, np.float32)

    nc, patterns = _get_program(mask)
    patflat = np.ascontiguousarray(patterns.reshape(-1, 128))

    def tr_tiled(x, dt):
        # [T, E] f32 -> partition-major [128, NJ*EC*TJ]: partition p holds
        # x^T[e*128+p, j*TJ+c] at col ((j*EC+e)*TJ + c) -> contiguous 1-packet
        # per-partition DMAs
        xt = np.ascontiguousarray(x.astype(dt).T)          # [E, T]
        xt = xt.reshape(EC, 128, NJ, TJ).transpose(1, 2, 0, 3)  # [128, NJ, EC, TJ]
        return np.ascontiguousarray(xt.reshape(128, NJ * EC * TJ))

    def wcat(W, half, dt, scale=1.0):
        # [H, E, D] -> partition-major [128, EC*DC] (e-chunk c at cols c*DC)
        w = (W[half * HPC : (half + 1) * HPC].transpose(1, 0, 2)
             .reshape(E, DC) * scale).astype(dt)
        w = w.reshape(EC, 128, DC).transpose(1, 0, 2)
        return np.ascontiguousarray(w.reshape(128, EC * DC))

    in_maps = []
    for c in range(8):
        b, half = divmod(c, 2)
        off = half * DC
        in_maps.append(
            {
                "qTt": tr_tiled(q[b], NPFP8),
                "kTt": tr_tiled(k[b], NPFP8),
                "vTt": tr_tiled(v[b], NPBF16),
                # wq/wk pre-scaled x16 for fp8 range; exp scale divides it out
                "wq": wcat(Wq, half, NPFP8, 16.0),
                "wk": wcat(Wk, half, NPFP8, 16.0),
                "wv": wcat(Wv, half, NPBF16),
                "wpT": np.ascontiguousarray(Wp[:, off : off + DC].T).astype(NPBF16),
                "pat": patflat,
                "selbc": _SELBC,
            }
        )
    return nc, in_maps


def kernel(k, q, v, mask, Wk, Wq, Wv, Wp, bp):
    bp = np.asarray(bp, np.float32)
    nc, in_maps = _prepare(k, q, v, mask, Wk, Wq, Wv, Wp)
    res = run_bass_kernel_spmd(nc, in_maps, list(range(8)))
    out = np.empty((B, T, E), np.float32)
    for b in range(B):
        yt = res.results[2 * b]["yTt"] + res.results[2 * b + 1]["yTt"]
        # [NJ*EC*128, TJ] -> [E, T]
        yt = yt.reshape(NJ, EC, 128, TJ).transpose(1, 2, 0, 3).reshape(E, T)
        out[b] = yt.T + bp[None, :]
    return out
